# revision 1
# baseline (speedup 1.0000x reference)
"""Bass/Tile kernel for nn_BaselineModel (gumbel matching + attention MLPs).

Layouts:
  - MLPs run in [feature-on-partition, row-on-free] layout, bf16 matmuls, f32 psum.
  - Matching (rho/argmax scan) runs in [batch-on-partition, (j,i)-free] layout, f32.
  - Per-batch f32 self-gram Z Z^T with Z=[ff;fo] (stationary=moving=[128,32] chunk)
    yields rho^T block and both squared norms (diagonals) in one accumulation group.
"""
import sys
sys.path.insert(0, "/opt/trn_rl_repo")
from contextlib import ExitStack
import concourse.bass as bass
import concourse.mybir as mybir
from concourse.masks import make_identity

F32 = mybir.dt.float32
BF16 = mybir.dt.bfloat16
AF = mybir.ActivationFunctionType
ALU = mybir.AluOpType
AX = mybir.AxisListType

NB = 16    # objects per batch
DV = 256   # visual feature dim
DL = 1024  # instruction dim


def ap_view(ap, dims, extra_offset=0):
    return bass.AP(tensor=ap.tensor, offset=ap.offset + extra_offset, ap=list(dims))


def pe_transpose(nc, out, in_, ident):
    return nc.tensor.matmul(out, in_, ident, is_transpose=True, start=True, stop=True)


def build_kernel(tc, io, BL):
    """io: dict name -> DRAM AP (inputs + out_pred, out_matched). BL: batches/core."""
    nc = tc.nc
    assert BL % 32 == 0
    SUB = 32                    # gram sub-block (batches)
    PB = min(128, BL)           # scan block (batches)
    NBLK = BL // PB
    NSUB_B = PB // SUB          # subs per block
    SUBG = SUB // 8             # groups of 8 batches per sub
    RT = 512                    # rows per attention tile
    BRT = RT // NB              # batches per attention row tile (32)
    NRT_B = PB // BRT           # row tiles per block

    ctx = ExitStack()

    # ---------- persistent pools ----------
    wpool = ctx.enter_context(tc.tile_pool(name="wpool", bufs=1))
    act = ctx.enter_context(tc.tile_pool(name="act", bufs=1))
    hp_big = ctx.enter_context(tc.tile_pool(name="hp_big", bufs=6))
    sm = ctx.enter_context(tc.tile_pool(name="sm", bufs=2))
    scp = ctx.enter_context(tc.tile_pool(name="scp", bufs=2))
    ps_mm = ctx.enter_context(tc.tile_pool(name="ps_mm", bufs=5, space="PSUM"))
    ps_tr = ctx.enter_context(tc.tile_pool(name="ps_tr", bufs=2, space="PSUM"))
    ps_gr = ctx.enter_context(tc.tile_pool(name="ps_gr", bufs=1, space="PSUM"))

    ident = wpool.tile([128, 128], F32)
    make_identity(nc, ident)
    ones = wpool.tile([16, 16], F32)
    nc.vector.memset(ones, 1.0)

    def load_bias_col(pool, name, M, k=None):
        """DRAM [M] (or [4,M] row k) -> [p, mb] tile, column m = feats m*128..."""
        mb = (M + 127) // 128
        p = min(M, 128)
        t = pool.tile([128, mb], F32, tag=f"b_{name}{'' if k is None else k}")
        off = 0 if k is None else k * M
        nc.sync.dma_start(out=t[:p, :], in_=ap_view(io[name], [[1, p], [128, mb]], off))
        return t

    # ================= phase A: weights for dec/map (scoped) =================
    with tc.tile_pool(name="wdec", bufs=1) as wdec, \
         tc.tile_pool(name="fip", bufs=2) as fip:

        def load_w(pool, name, K, M, k=None):
            kc = (K + 127) // 128
            p = min(K, 128)
            t = pool.tile([128, kc, M], BF16, tag=f"w_{name}{'' if k is None else k}")
            src = io[name] if k is None else io[name][k]
            view = src.rearrange("(c p) m -> p c m", p=128) if K >= 128 else src.unsqueeze(1)
            nc.gpsimd.dma_start(out=t[:p], in_=view)
            return t

        # ---------- f_instruction -> finstT bf16 [128, 8, BL] ----------
        finstT = act.tile([128, 8, BL], BF16)
        nbt = max(1, BL // 128)
        btr = min(BL, 128)
        for bt in range(nbt):
            fi = fip.tile([128, DL], F32, tag="fi")
            nc.sync.dma_start(out=fi[:btr], in_=io["f_instruction"][bt * 128:bt * 128 + btr])
            for kc in range(8):
                pt_full = ps_tr.tile([128, 512], F32, tag="tr")
                pt = pt_full[:, :128]
                pe_transpose(nc, pt[:, :btr], fi[:btr, kc * 128:(kc + 1) * 128], ident[:btr, :btr])
                nc.scalar.copy(finstT[:, kc, bt * 128:bt * 128 + btr], pt[:, :btr])
        fiT = [finstT[:, c, :] for c in range(8)]

        w_dec = [load_w(wdec, f"dec_W{i}", K, M) for i, (K, M) in
                 enumerate([(1024, 512), (512, 256), (256, 256), (256, 128), (128, 3)], 1)]
        b_dec = [load_bias_col(wdec, f"dec_b{i}", M) for i, M in
                 enumerate([512, 256, 256, 128], 1)]
        b_dec5 = load_bias_col(wpool, "dec_b5", 3)
        w_map1 = [load_w(wdec, "map_W1", 1024, 512, k) for k in range(4)]
        w_map2 = [load_w(wdec, "map_W2", 512, 256, k) for k in range(4)]
        b_map1 = [load_bias_col(wdec, "map_b1", 512, k) for k in range(4)]
        b_map2 = [load_bias_col(wdec, "map_b2", 256, k) for k in range(4)]
        # att/pred weights persist (loaded here to reuse wstg staging)
        w_att1 = [load_w(wpool, "att_W1", 512, 256, k) for k in range(4)]
        w_att2 = [load_w(wpool, "att_W2", 256, 256, k) for k in range(4)]
        w_att3 = [load_w(wpool, "att_W3", 256, 128, k) for k in range(4)]
        w_att4 = [load_w(wpool, "att_W4", 128, 1, k) for k in range(4)]
        b_att1 = [load_bias_col(wpool, "att_b1", 256, k) for k in range(4)]
        b_att2 = [load_bias_col(wpool, "att_b2", 256, k) for k in range(4)]
        b_att3 = [load_bias_col(wpool, "att_b3", 128, k) for k in range(4)]
        b_att4 = [load_bias_col(wpool, "att_b4", 1, k) for k in range(4)]
        w_pred = [load_w(wpool, f"pred_W{i}", K, M) for i, (K, M) in
                  enumerate([(13, 256), (256, 512), (512, 512), (512, 256), (256, 5)], 1)]
        b_pred = [load_bias_col(wpool, f"pred_b{i}", M) for i, M in
                  enumerate([256, 512, 512, 256], 1)]
        b_pred5 = load_bias_col(wpool, "pred_b5", 5)

        def mm_block(x_tiles, w, Kin, Mout, m, n_cols):
            kc = max(1, Kin // 128)
            kp = min(128, Kin)
            mw = min(128, Mout - m * 128)
            ps_full = ps_mm.tile([128, 512], F32, tag="mm")
            ps = ps_full[:, :n_cols]
            for c in range(kc):
                nc.tensor.matmul(ps[:mw], w[:kp, c, m * 128:m * 128 + mw],
                                 x_tiles[c][:kp], start=(c == 0), stop=(c == kc - 1))
            return ps, mw

        # ---------- dec MLP -> f_action -> emb[0:3] ----------
        emb = act.tile([16, BL], BF16)
        h = fiT
        for li, (K, M) in enumerate([(1024, 512), (512, 256), (256, 256), (256, 128)]):
            nh = []
            for m in range((M + 127) // 128):
                ps, mw = mm_block(h, w_dec[li], K, M, m, BL)
                o = hp_big.tile([128, BL], BF16, tag="dec")
                nc.scalar.activation(o[:mw], ps[:mw], AF.Relu, bias=b_dec[li][:mw, m:m + 1])
                nh.append(o)
            h = nh
        ps5, _ = mm_block(h, w_dec[4], 128, 3, 0, BL)
        e_sb = sm.tile([3, BL], F32, tag="e_sb")
        nc.scalar.activation(e_sb, ps5[:3], AF.Exp, bias=b_dec5[:3, 0:1])
        ps_s_full = ps_mm.tile([128, 512], F32, tag="mm")
        ps_s = ps_s_full[:, :BL]
        nc.tensor.matmul(ps_s[:1], ones[:3, 0:1], e_sb[:], start=True, stop=True)
        r_sb = sm.tile([1, BL], F32, tag="r_sb")
        nc.vector.reciprocal(r_sb, ps_s[:1])
        ps_rb_full = ps_mm.tile([128, 512], F32, tag="mm")
        ps_rb = ps_rb_full[:, :BL]
        nc.tensor.matmul(ps_rb[:3], ones[0:1, 0:3], r_sb[:], start=True, stop=True)
        nc.vector.tensor_tensor(out=emb[0:3], in0=e_sb[:], in1=ps_rb[:3], op=ALU.mult)

        # ---------- map MLPs -> c_k (att1 m-contribution + b1), f32 ----------
        c_k = []
        for k in range(4):
            h1 = []
            for m in range(4):
                ps, mw = mm_block(fiT, w_map1[k], 1024, 512, m, BL)
                o = hp_big.tile([128, BL], BF16, tag="map1")
                nc.scalar.activation(o, ps, AF.Relu, bias=b_map1[k][:, m:m + 1])
                h1.append(o)
            mk = []
            for m in range(2):
                ps, mw = mm_block(h1, w_map2[k], 512, 256, m, BL)
                o = hp_big.tile([128, BL], BF16, tag="mk")
                nc.vector.tensor_scalar_add(o, ps, b_map2[k][:, m:m + 1])
                mk.append(o)
            cks = []
            for m in range(2):
                # att_W1 rows 0:256 (kc chunks 0,1) are the m-part
                kcs = [0, 1]
                ps_full = ps_mm.tile([128, 512], F32, tag="mm")
                ps = ps_full[:, :BL]
                for j, c in enumerate(kcs):
                    nc.tensor.matmul(ps, w_att1[k][:, c, m * 128:(m + 1) * 128],
                                     mk[j][:], start=(j == 0), stop=(j == 1))
                ckt = act.tile([128, BL], F32, tag=f"ck{k}_{m}")
                nc.vector.tensor_scalar_add(ckt, ps, b_att1[k][:, m:m + 1])
                cks.append(ckt)
            c_k.append(cks)

    # ================= phase B: per-block gram+scan+attention =================
    out_pred = io["out_pred"]
    out_matched = io["out_matched"]
    blk_state = []

    with tc.tile_pool(name="tp", bufs=3) as tp, \
         tc.tile_pool(name="zp", bufs=2) as zp, \
         tc.tile_pool(name="h1p", bufs=10) as h1p, \
         tc.tile_pool(name="h2p", bufs=10) as h2p, \
         tc.tile_pool(name="h3p", bufs=3) as h3p:

        foT_blks = []
        for blk in range(NBLK):
            b0 = blk * PB
            scan_gram = scp.tile([128, 32, 32], F32, tag="scan_gram")
            foT = act.tile([128, 2, PB * NB], BF16, tag=f"foT{blk}")
            foT_blks.append(foT)
            for si in range(NSUB_B):
                s0 = b0 + si * SUB
                fo_nat = tp.tile([128, SUBG, 256], F32, tag="fo_nat")
                ff_nat = tp.tile([128, SUBG, 256], F32, tag="ff_nat")
                nc.sync.dma_start(out=fo_nat, in_=io["f_objects"][s0:s0 + SUB]
                                  .rearrange("(g b) i d -> (b i) g d", b=8))
                nc.sync.dma_start(out=ff_nat, in_=io["f_objects_final"][s0:s0 + SUB]
                                  .rearrange("(g b) i d -> (b i) g d", b=8))
                zt = zp.tile([128, SUB, 2, 32], F32, tag="zt")
                for g in range(SUBG):
                    for c in range(2):
                        ptf_full = ps_tr.tile([128, 512], F32, tag="tr")
                        ptf = ptf_full[:, :128]
                        pe_transpose(nc, ptf, ff_nat[:, g, c * 128:(c + 1) * 128], ident)
                        nc.vector.tensor_copy(zt[:, g * 8:g * 8 + 8, c, 0:16],
                                              ptf.rearrange("p (b j) -> p b j", b=8))
                        pto_full = ps_tr.tile([128, 512], F32, tag="tr")
                        pto = pto_full[:, :128]
                        pe_transpose(nc, pto, fo_nat[:, g, c * 128:(c + 1) * 128], ident)
                        nc.vector.tensor_copy(zt[:, g * 8:g * 8 + 8, c, 16:32],
                                              pto.rearrange("p (b j) -> p b j", b=8))
                        col = (si * SUB + g * 8) * NB
                        nc.scalar.copy(foT[:, c, col:col + 128], pto)
                gram_sb = sm.tile([128, SUB // 4, 32], F32, tag="gram_sb")
                for q in range(SUB // 4):
                    gp_full = ps_gr.tile([128, 512], F32, tag="gram")
                    gp = gp_full[:, :32]
                    for bi_ in range(4):
                        bb = q * 4 + bi_
                        for c in range(2):
                            nc.tensor.matmul(gp[bi_ * 32:bi_ * 32 + 32], zt[:, bb, c, :],
                                             zt[:, bb, c, :], start=(c == 0), stop=(c == 1),
                                             tile_position=(0, bi_ * 32))
                    nc.scalar.copy(gram_sb[:, q, :], gp)
                for q in range(SUB // 4):
                    p0 = si * SUB + q * 4
                    eng = nc.sync if q % 2 == 0 else nc.gpsimd
                    eng.dma_start(out=scan_gram[p0:p0 + 4], in_=gram_sb[:, q, :])

            # ---------- matching scan for this block (DVE) ----------
            gum = scp.tile([128, NB, NB], F32, tag="gum")
            nc.sync.dma_start(out=gum[:PB], in_=io["gumbel"][b0:b0 + PB])
            gT = scp.tile([128, NB, NB], F32, tag="gT")
            nc.vector.tensor_copy(gT[:PB], gum[:PB].transpose([0, 2, 1]))
            bfx = scp.tile([128, NB, 5], F32, tag="bfx")
            nc.sync.dma_start(out=bfx[:PB], in_=io["bboxes_f"][b0:b0 + PB])
            bix = act.tile([128, NB, 5], F32, tag=f"bix{blk}")
            nc.sync.dma_start(out=bix[:PB], in_=io["bboxes_i"][b0:b0 + PB])

            gflat = scan_gram.rearrange("p a b -> p (a b)")
            sq_ff = sm.tile([128, NB], F32, tag="sq_ff")
            nc.scalar.activation(sq_ff[:PB], ap_view(gflat, [list(gflat.ap[0]), [33, 16]])[:PB],
                                 AF.Sqrt)
            inv_ff = sm.tile([128, NB], F32, tag="inv_ff")
            nc.vector.reciprocal(inv_ff[:PB], sq_ff[:PB])
            sq_fo = sm.tile([128, NB], F32, tag="sq_fo")
            nc.scalar.activation(sq_fo[:PB], ap_view(gflat, [list(gflat.ap[0]), [33, 16]], 528)[:PB],
                                 AF.Sqrt)
            inv_fo = sm.tile([128, NB], F32, tag="inv_fo")
            nc.vector.reciprocal(inv_fo[:PB], sq_fo[:PB])

            base = scp.tile([128, NB, NB], F32, tag="base")  # [p, j, i]
            nc.vector.tensor_tensor(out=base[:PB], in0=scan_gram[:PB, 0:16, 16:32],
                                    in1=inv_ff[:PB].unsqueeze(2).broadcast_to([PB, 16, 16]),
                                    op=ALU.mult)
            nc.vector.tensor_tensor(out=base[:PB], in0=base[:PB],
                                    in1=inv_fo[:PB].unsqueeze(1).broadcast_to([PB, 16, 16]),
                                    op=ALU.mult)
            nc.vector.tensor_tensor(out=base[:PB], in0=base[:PB], in1=gT[:PB], op=ALU.add)

            pen = sm.tile([128, NB], F32, tag="pen")
            nc.vector.memset(pen[:PB], 0.0)
            boxes = scp.tile([128, NB, 5], F32, tag="boxes")
            for i in range(NB):
                score = sm.tile([128, NB], F32, tag="score")
                nc.vector.tensor_tensor(out=score[:PB], in0=base[:PB, :, i], in1=pen[:PB],
                                        op=ALU.add)
                mx8 = sm.tile([128, 8], F32, tag="mx8")
                nc.vector.max(mx8[:PB], score[:PB])
                oh = sm.tile([128, NB], F32, tag="oh")
                nc.vector.tensor_scalar(out=oh[:PB], in0=score[:PB], scalar1=mx8[:PB, 0:1],
                                        scalar2=None, op0=ALU.is_equal)
                nc.vector.scalar_tensor_tensor(out=pen[:PB], in0=oh[:PB], scalar=-1e5,
                                               in1=pen[:PB], op0=ALU.mult, op1=ALU.add)
                prod = sm.tile([128, 5, NB], F32, tag="prod")
                nc.vector.tensor_tensor(out=prod[:PB], in0=bfx[:PB].transpose([0, 2, 1]),
                                        in1=oh[:PB].unsqueeze(1).broadcast_to([PB, 5, NB]),
                                        op=ALU.mult)
                nc.vector.reduce_sum(boxes[:PB, i, :], prod[:PB], axis=AX.X)
            nc.sync.dma_start(out=out_matched[b0:b0 + PB], in_=boxes[:PB])

            blk_state.append(bix)

        for blk in range(NBLK):
            b0 = blk * PB
            foT = foT_blks[blk]
            bix = blk_state[blk]
            # ---------- attention for this block ----------
            sT = act.tile([128, 4, NB], F32, tag=f"sT{blk}")
            for rt in range(NRT_B):
                col0 = rt * RT
                gb0 = b0 + rt * BRT  # global batch of first row
                for k in range(4):
                    h1 = []
                    for m in range(2):
                        ps_full = ps_mm.tile([128, 512], F32, tag="mm")
                        ps = ps_full[:, :RT]
                        for c in range(2):
                            nc.tensor.matmul(ps, w_att1[k][:, 2 + c, m * 128:(m + 1) * 128],
                                             foT[:, c, col0:col0 + RT],
                                             start=(c == 0), stop=(c == 1))
                        o = h1p.tile([128, RT], BF16, tag="h1")
                        nc.vector.tensor_tensor(
                            out=o.rearrange("p (b n) -> p b n", n=NB),
                            in0=ps.rearrange("p (b n) -> p b n", n=NB),
                            in1=c_k[k][m][:, gb0:gb0 + BRT].unsqueeze(2)
                                .broadcast_to([128, BRT, NB]),
                            op=ALU.add)
                        nc.gpsimd.tensor_scalar_max(o, o, 0.0)
                        h1.append(o)
                    h2 = []
                    for m in range(2):
                        ps_full = ps_mm.tile([128, 512], F32, tag="mm")
                        ps = ps_full[:, :RT]
                        for c in range(2):
                            nc.tensor.matmul(ps, w_att2[k][:, c, m * 128:(m + 1) * 128],
                                             h1[c][:], start=(c == 0), stop=(c == 1))
                        o = h2p.tile([128, RT], BF16, tag="h2")
                        nc.scalar.activation(o, ps, AF.Relu, bias=b_att2[k][:, m:m + 1])
                        h2.append(o)
                    ps_full = ps_mm.tile([128, 512], F32, tag="mm")
                    ps = ps_full[:, :RT]
                    for c in range(2):
                        nc.tensor.matmul(ps, w_att3[k][:, c, 0:128], h2[c][:],
                                         start=(c == 0), stop=(c == 1))
                    h3 = h3p.tile([128, RT], BF16, tag="h3")
                    nc.scalar.activation(h3, ps, AF.Relu, bias=b_att3[k][:, 0:1])
                    ps4_full = ps_mm.tile([128, 512], F32, tag="mm")
                    ps4 = ps4_full[:, :RT]
                    nc.tensor.matmul(ps4[:1], w_att4[k][:, 0, 0:1], h3[:], start=True, stop=True)
                    s_rt = sm.tile([1, RT], F32, tag="s_rt")
                    nc.vector.tensor_scalar_add(s_rt, ps4[:1], b_att4[k][0:1, 0:1])
                    eng = nc.sync if k % 2 == 0 else nc.gpsimd
                    eng.dma_start(out=sT[rt * BRT:(rt + 1) * BRT, k, :],
                                  in_=ap_view(s_rt, [[1, 1], [NB, BRT], [1, NB]]))

            # ---------- softmax over objects, locs, emb rows ----------
            def softmax_pair(ka, kb, tag):
                z = sm.tile([128, NB], F32, tag=f"z{tag}")
                nc.vector.tensor_tensor(out=z[:PB], in0=sT[:PB, ka, :], in1=sT[:PB, kb, :],
                                        op=ALU.add)
                e = sm.tile([128, NB], F32, tag=f"e{tag}")
                nc.scalar.activation(e[:PB], z[:PB], AF.Exp)
                ssum = sm.tile([128, 1], F32, tag=f"ss{tag}")
                nc.vector.reduce_sum(ssum[:PB], e[:PB], axis=AX.X)
                rinv = sm.tile([128, 1], F32, tag=f"ri{tag}")
                nc.vector.reciprocal(rinv[:PB], ssum[:PB])
                a = act.tile([128, NB], F32, tag=f"a{tag}{blk}")
                nc.vector.tensor_scalar(out=a[:PB], in0=e[:PB], scalar1=rinv[:PB, 0:1],
                                        scalar2=None, op0=ALU.mult)
                return a

            a_sub = softmax_pair(0, 1, "s")
            a_obj = softmax_pair(2, 3, "o")

            def weighted_loc(a, tag):
                prod = sm.tile([128, 5, NB], F32, tag=f"lp{tag}")
                nc.vector.tensor_tensor(out=prod[:PB], in0=bix[:PB].transpose([0, 2, 1]),
                                        in1=a[:PB].unsqueeze(1).broadcast_to([PB, 5, NB]),
                                        op=ALU.mult)
                loc = sm.tile([128, 5], F32, tag=f"loc{tag}")
                nc.vector.reduce_sum(loc[:PB], prod[:PB], axis=AX.X)
                return loc

            sloc = weighted_loc(a_sub, "s")
            oloc = weighted_loc(a_obj, "o")
            for loc, r0 in ((oloc, 3), (sloc, 8)):
                pt_full = ps_tr.tile([128, 512], F32, tag="tr")
                pt = pt_full[:, :128]
                pe_transpose(nc, pt[:5, :PB], loc[:PB], ident[:PB, :PB])
                locT = sm.tile([5, 128], BF16, tag="locT")
                nc.scalar.copy(locT[:, :PB], pt[:5, :PB])
                nc.sync.dma_start(out=emb[r0:r0 + 5, b0:b0 + PB], in_=locT[:, :PB])
            blk_state[blk] = (a_sub, bix)

        # ================= phase C: pred MLP + blend =================
        h = [emb]
        dims = [(13, 256), (256, 512), (512, 512), (512, 256)]
        for li, (K, M) in enumerate(dims):
            kc = max(1, K // 128)
            kp = min(128, K)
            nh = []
            for m in range((M + 127) // 128):
                ps_full = ps_mm.tile([128, 512], F32, tag="mm")
                ps = ps_full[:, :BL]
                for c in range(kc):
                    nc.tensor.matmul(ps, w_pred[li][:kp, c, m * 128:(m + 1) * 128],
                                     h[c][:kp], start=(c == 0), stop=(c == kc - 1))
                o = hp_big.tile([128, BL], BF16, tag="pred")
                nc.scalar.activation(o, ps, AF.Relu, bias=b_pred[li][:, m:m + 1])
                nh.append(o)
            h = nh
        ps5_full = ps_mm.tile([128, 512], F32, tag="mm")
        ps5 = ps5_full[:, :BL]
        for c in range(2):
            nc.tensor.matmul(ps5[:5], w_pred[4][:, c, 0:5], h[c][:], start=(c == 0), stop=(c == 1))
        predv = act.tile([5, BL], F32)
        nc.scalar.activation(predv, ps5[:5], AF.Tanh, bias=b_pred5[:5, 0:1])

        for blk in range(NBLK):
            b0 = blk * PB
            a_sub, bix = blk_state[blk]
            pt_full = ps_tr.tile([128, 512], F32, tag="tr")
            pt = pt_full[:, :128]
            pe_transpose(nc, pt[:PB, :5], predv[:, b0:b0 + PB], ident[:5, :5])
            predT = sm.tile([128, 5], F32, tag="predT")
            nc.vector.tensor_copy(predT[:PB], pt[:PB, :5])
            d = scp.tile([128, NB, 5], F32, tag="d")
            nc.vector.tensor_tensor(out=d[:PB], in0=predT[:PB].unsqueeze(1)
                                    .broadcast_to([PB, NB, 5]), in1=bix[:PB], op=ALU.subtract)
            nc.vector.tensor_tensor(out=d[:PB], in0=d[:PB],
                                    in1=a_sub[:PB].unsqueeze(2).broadcast_to([PB, NB, 5]),
                                    op=ALU.mult)
            outb = scp.tile([128, NB, 5], F32, tag="outb")
            nc.vector.tensor_tensor(out=outb[:PB], in0=d[:PB], in1=bix[:PB], op=ALU.add)
            nc.sync.dma_start(out=out_pred[b0:b0 + PB], in_=outb[:PB])

    ctx.close()


INPUT_SPECS = [
    ("f_objects", (NB, DV)), ("f_objects_final", (NB, DV)),
    ("bboxes_i", (NB, 5)), ("bboxes_f", (NB, 5)),
    ("f_instruction", (DL,)), ("gumbel", (NB, NB)),
]
WEIGHT_SPECS = (
    [(f"dec_W{i}", s) for i, s in enumerate([(1024, 512), (512, 256), (256, 256), (256, 128), (128, 3)], 1)]
    + [(f"dec_b{i}", (s,)) for i, s in enumerate([512, 256, 256, 128, 3], 1)]
    + [("map_W1", (4, 1024, 512)), ("map_b1", (4, 512)), ("map_W2", (4, 512, 256)), ("map_b2", (4, 256))]
    + [(f"att_W{i}", (4,) + s) for i, s in enumerate([(512, 256), (256, 256), (256, 128), (128, 1)], 1)]
    + [(f"att_b{i}", (4, s)) for i, s in enumerate([256, 256, 128, 1], 1)]
    + [(f"pred_W{i}", s) for i, s in enumerate([(13, 256), (256, 512), (512, 512), (512, 256), (256, 5)], 1)]
    + [(f"pred_b{i}", (s,)) for i, s in enumerate([256, 512, 512, 256, 5], 1)]
)


def declare_io(nc, BL):
    io = {}
    for name, tail in INPUT_SPECS:
        io[name] = nc.dram_tensor(name, [BL] + list(tail), F32, kind="ExternalInput").ap()
    for name, shape in WEIGHT_SPECS:
        io[name] = nc.dram_tensor(name, list(shape), F32, kind="ExternalInput").ap()
    io["out_pred"] = nc.dram_tensor("out_pred", [BL, NB, 5], F32, kind="ExternalOutput").ap()
    io["out_matched"] = nc.dram_tensor("out_matched", [BL, NB, 5], F32, kind="ExternalOutput").ap()
    return io
# ======================= SPMD driver =======================
import numpy as np

N_CORES = 8
B_FULL = 4096
BL_CORE = B_FULL // N_CORES

_BATCH_INPUTS = ("f_objects", "f_objects_final", "bboxes_i", "bboxes_f",
                 "f_instruction", "gumbel")

_NC = None


def _get_nc():
    global _NC
    if _NC is None:
        from concourse import bacc
        import concourse.tile as tile
        nc = bacc.Bacc("TRN2", target_bir_lowering=False, debug=False,
                       num_devices=N_CORES)
        io = declare_io(nc, BL_CORE)
        with tile.TileContext(nc) as tc:
            build_kernel(tc, io, BL_CORE)
        nc.compile()
        _NC = nc
    return _NC


def kernel(**inputs):
    from concourse.bass_utils import run_bass_kernel_spmd
    nc = _get_nc()
    arrs = {k: np.ascontiguousarray(np.asarray(v, dtype=np.float32))
            for k, v in inputs.items()}
    in_maps = []
    for c in range(N_CORES):
        m = {}
        for k, v in arrs.items():
            if k in _BATCH_INPUTS:
                m[k] = v[c * BL_CORE:(c + 1) * BL_CORE]
            else:
                m[k] = v
        in_maps.append(m)
    res = run_bass_kernel_spmd(nc, in_maps, list(range(N_CORES)))
    pred = np.concatenate([res.results[c]["out_pred"] for c in range(N_CORES)], axis=0)
    matched = np.concatenate([res.results[c]["out_matched"] for c in range(N_CORES)], axis=0)
    return pred, matched



# revision 19
# speedup vs baseline: 759.2449x; 759.2449x over previous
"""Bass/Tile kernel for nn_BaselineModel (gumbel matching + attention MLPs).

v2 layout/precision plan:
  - MLPs in [feature-on-partition, row-on-free], fp8e4 operands with DoubleRow
    matmuls (K>=256), f32 psum. Matching stays f32 (argmax-flip safety).
  - Per-batch map-branch contribution c_k enters the att1 PSUM group via a
    K=32 one-hot "mask matmul" (batch -> 16 rows), so h1 is a single ACT
    Relu from PSUM (no gpsimd, no DVE broadcast add).
  - att4 scores for the 4 branches land in one PSUM bank at partitions
    {0,32,64,96} (tile_position col groups); one copy + one DMA per (k,blk)
    delivers contiguous [batch, object] score tiles. b_att4 is dropped
    (softmax shift invariance).
  - Matching scan runs on the transposed rho block (free transpose from the
    symmetric Z Z^T gram), so gumbel needs no transpose and every DVE op in
    the scan is contiguous. Matched boxes = accumulated one-hot matrix
    applied post-loop.
"""
import sys
sys.path.insert(0, "/opt/trn_rl_repo")
from contextlib import ExitStack
import concourse.bass as bass
import concourse.mybir as mybir
from concourse.masks import make_identity

F32 = mybir.dt.float32
BF16 = mybir.dt.bfloat16
FP8 = mybir.dt.float8e4
AF = mybir.ActivationFunctionType
ALU = mybir.AluOpType
AX = mybir.AxisListType
DR = mybir.MatmulPerfMode.DoubleRow

NB = 16    # objects per batch
DV = 256   # visual feature dim
DL = 1024  # instruction dim


def ap_view(ap, dims, extra_offset=0):
    return bass.AP(tensor=ap.tensor, offset=ap.offset + extra_offset, ap=list(dims))


def pe_transpose(nc, out, in_, ident):
    return nc.tensor.matmul(out, in_, ident, is_transpose=True, start=True, stop=True)


def build_kernel(tc, io, BL):
    """io: dict name -> DRAM AP (inputs + out_pred, out_matched). BL: batches/core."""
    nc = tc.nc
    assert BL % 32 == 0
    SUB = 32                    # gram sub-block (batches)
    PB = min(128, BL)           # scan block (batches)
    NBLK = BL // PB
    NSUB_B = PB // SUB          # subs per block
    SUBG = SUB // 8             # groups of 8 batches per sub
    RT = 512                    # rows per attention tile
    BRT = RT // NB              # batches per attention row tile (32)
    NRT_B = PB // BRT           # row tiles per block

    ctx = ExitStack()

    # ---------- persistent pools ----------
    wpool = ctx.enter_context(tc.tile_pool(name="wpool", bufs=1))
    act = ctx.enter_context(tc.tile_pool(name="act", bufs=1))
    hp_big = ctx.enter_context(tc.tile_pool(name="hp_big", bufs=6))
    sm = ctx.enter_context(tc.tile_pool(name="sm", bufs=2))
    scp = ctx.enter_context(tc.tile_pool(name="scp", bufs=2))
    ps_mm = ctx.enter_context(tc.tile_pool(name="ps_mm", bufs=2, space="PSUM"))
    ps_sm = ctx.enter_context(tc.tile_pool(name="ps_sm", bufs=2, space="PSUM"))
    ps_tr = ctx.enter_context(tc.tile_pool(name="ps_tr", bufs=2, space="PSUM"))

    ident = wpool.tile([128, 128], F32)
    make_identity(nc, ident)
    ones = wpool.tile([16, 16], F32)
    nc.vector.memset(ones, 1.0)
    ones1 = wpool.tile([1, 128], FP8)
    nc.vector.memset(ones1, 1.0)
    # one-hot batch->row expansion: mask128[32g + j, b*NB + n] = (b == j)
    mask128 = wpool.tile([128, 32, NB], BF16)
    for g in range(4):
        nc.vector.tensor_copy(
            mask128[g * 32:(g + 1) * 32],
            ident[g * 32:(g + 1) * 32, g * 32:(g + 1) * 32].unsqueeze(2)
            .broadcast_to([32, 32, NB]))

    def load_bias_col(pool, name, M, k=None):
        """DRAM [M] (or [4,M] row k) -> [p, mb] tile, column m = feats m*128..."""
        mb = (M + 127) // 128
        p = min(M, 128)
        t = pool.tile([128, mb], F32, tag=f"b_{name}{'' if k is None else k}")
        off = 0 if k is None else k * M
        nc.sync.dma_start(out=t[:p, :], in_=ap_view(io[name], [[1, p], [128, mb]], off))
        return t

    def load_bias_row(pool, name, M, k=None):
        """DRAM [M] (or [4,M] row k) -> [1, M] row tile."""
        t = pool.tile([1, M], FP8, tag=f"br_{name}{'' if k is None else k}")
        off = 0 if k is None else k * M
        nc.gpsimd.dma_start(out=t, in_=ap_view(io[name], [[M, 1], [1, M]], off))
        return t

    # ================= phase A: weights for dec/map (scoped) =================
    with tc.tile_pool(name="wdec", bufs=1) as wdec, \
         tc.tile_pool(name="fip", bufs=2) as fip:

        def load_w(pool, name, K, M, k=None, dt=FP8):
            kc = (K + 127) // 128
            p = min(K, 128)
            t = pool.tile([128, kc, M], dt, tag=f"w_{name}{'' if k is None else k}")
            src = io[name] if k is None else io[name][k]
            view = src.rearrange("(c p) m -> p c m", p=128) if K >= 128 else src.unsqueeze(1)
            nc.gpsimd.dma_start(out=t[:p], in_=view)
            return t

        # ---------- f_instruction -> finstT fp8 [128, 8, BL] ----------
        finstT = act.tile([128, 8, BL], FP8)
        nbt = max(1, BL // 128)
        btr = min(BL, 128)
        for bt in range(nbt):
            fi = fip.tile([128, DL], F32, tag="fi")
            nc.sync.dma_start(out=fi[:btr], in_=io["f_instruction"][bt * 128:bt * 128 + btr])
            for kc in range(8):
                pt_full = ps_tr.tile([128, 512], F32, tag="tr")
                pt = pt_full[:, :128]
                pe_transpose(nc, pt[:, :btr], fi[:btr, kc * 128:(kc + 1) * 128], ident[:btr, :btr])
                if kc % 2 == 0:
                    nc.scalar.copy(finstT[:, kc, bt * 128:bt * 128 + btr], pt[:, :btr])
                else:
                    nc.vector.tensor_copy(finstT[:, kc, bt * 128:bt * 128 + btr], pt[:, :btr])

        w_dec = [load_w(wdec, f"dec_W{i}", K, M) for i, (K, M) in
                 enumerate([(1024, 512), (512, 256), (256, 256), (256, 128), (128, 3)], 1)]
        b_dec = [load_bias_col(wdec, f"dec_b{i}", M) for i, M in
                 enumerate([512, 256, 256, 128], 1)]
        b_dec5 = load_bias_col(wpool, "dec_b5", 3)
        w_map1 = [load_w(wdec, "map_W1", 1024, 512, k) for k in range(4)]
        w_map2 = [load_w(wdec, "map_W2", 512, 256, k) for k in range(4)]
        b_map1 = [load_bias_col(wdec, "map_b1", 512, k) for k in range(4)]
        b_map2 = [load_bias_col(wdec, "map_b2", 256, k) for k in range(4)]
        # att/pred weights persist
        w_att1 = [load_w(wpool, "att_W1", 512, 256, k) for k in range(4)]
        w_att2 = [load_w(wpool, "att_W2", 256, 256, k) for k in range(4)]
        w_att3 = [load_w(wpool, "att_W3", 256, 128, k) for k in range(4)]
        w_att4 = [load_w(wpool, "att_W4", 128, 1, k) for k in range(4)]
        b_att1r = [load_bias_row(wpool, "att_b1", 256, k) for k in range(4)]
        b_att2 = [load_bias_col(wpool, "att_b2", 256, k) for k in range(4)]
        b_att3 = [load_bias_col(wpool, "att_b3", 128, k) for k in range(4)]
        w_pred = [load_w(wpool, f"pred_W{i}", K, M) for i, (K, M) in
                  enumerate([(13, 256), (256, 512), (512, 512), (512, 256), (256, 5)], 1)]
        b_pred = [load_bias_col(wpool, f"pred_b{i}", M) for i, M in
                  enumerate([256, 512, 512, 256], 1)]
        b_pred5 = load_bias_col(wpool, "pred_b5", 5)

        def mm_dr(ps, w, x, kc, m, mw, n_lo, n_hi):
            """psum[:mw, :] += sum_c w[:,c,m128:+mw].T @ x[:,c,n_lo:n_hi], DoubleRow pairs."""
            kc2 = kc // 2
            for c2 in range(kc2):
                nc.tensor.matmul(ps[:mw], w[:, 2 * c2:2 * c2 + 2, m * 128:m * 128 + mw],
                                 x[:, 2 * c2:2 * c2 + 2, n_lo:n_hi], perf_mode=DR,
                                 start=(c2 == 0), stop=(c2 == kc2 - 1))

        def mlp_layer(pool, x, w, b, kc, M, act_fn=AF.Relu, tag="h"):
            """x: [128, kc, BL] fp8 -> out [128, M/128, BL] fp8 via DR matmuls + ACT."""
            mb = (M + 127) // 128
            o = pool.tile([128, mb, BL], FP8, tag=tag)
            for m in range(mb):
                mw = min(128, M - m * 128)
                ps_full = ps_mm.tile([128, 1024], F32, tag="mm")
                ps = ps_full[:, :BL]
                mm_dr(ps, w, x, kc, m, mw, 0, BL)
                if m % 2 == 0:
                    nc.scalar.activation(o[:mw, m, :], ps[:mw], act_fn, bias=b[:mw, m:m + 1])
                elif act_fn == AF.Relu:
                    nc.vector.tensor_scalar(out=o[:mw, m, :], in0=ps[:mw],
                                            scalar1=b[:mw, m:m + 1], scalar2=0.0,
                                            op0=ALU.add, op1=ALU.max)
                else:
                    nc.vector.tensor_scalar(out=o[:mw, m, :], in0=ps[:mw],
                                            scalar1=b[:mw, m:m + 1], scalar2=None,
                                            op0=ALU.add)
            return o

        # ---------- dec MLP -> f_action -> emb[0:3] ----------
        emb = act.tile([16, BL], FP8)
        h = finstT
        for li, (K, M) in enumerate([(1024, 512), (512, 256), (256, 256), (256, 128)]):
            h = mlp_layer(hp_big, h, w_dec[li], b_dec[li], K // 128, M, tag="dec")
        ps5_full = ps_mm.tile([128, 1024], F32, tag="mm")
        ps5 = ps5_full[:, :BL]
        nc.tensor.matmul(ps5[:3], w_dec[4][:, 0, 0:3], h[:, 0, :], start=True, stop=True)
        e_sb = sm.tile([3, BL], F32, tag="e_sb")
        nc.scalar.activation(e_sb, ps5[:3], AF.Exp, bias=b_dec5[:3, 0:1])
        ps_s_full = ps_sm.tile([128, 512], F32, tag="sm")
        ps_s = ps_s_full[:, :BL]
        nc.tensor.matmul(ps_s[:1], ones[:3, 0:1], e_sb[:], start=True, stop=True)
        r_sb = sm.tile([1, BL], F32, tag="r_sb")
        nc.vector.reciprocal(r_sb, ps_s[:1])
        ps_rb_full = ps_sm.tile([128, 512], F32, tag="sm")
        ps_rb = ps_rb_full[:, :BL]
        nc.tensor.matmul(ps_rb[:3], ones[0:1, 0:3], r_sb[:], start=True, stop=True)
        nc.vector.tensor_tensor(out=emb[0:3], in0=e_sb[:], in1=ps_rb[:3], op=ALU.mult)

        # ---------- map MLPs -> c_kT [batch, feat] bf16 (includes b_att1) ----------
        ckT = []
        for k in range(4):
            h1m = mlp_layer(hp_big, finstT, w_map1[k], b_map1[k], 8, 512, tag="map1")
            mk = mlp_layer(hp_big, h1m, w_map2[k], b_map2[k], 4, 256,
                           act_fn=AF.Identity, tag="mk")
            # ckt_t: [128 batch-in-block, nblk, 256 feat], includes b_att1
            ckt_t = act.tile([128, BL // 128, 256], BF16, tag=f"ckT{k}")
            for bblk in range(BL // 128):
                ps_full = ps_sm.tile([128, 512], F32, tag="sm")
                ps = ps_full[:, :256]
                nc.tensor.matmul(ps, mk[:, 0:2, bblk * 128:bblk * 128 + 128],
                                 w_att1[k][:, 0:2, 0:256], perf_mode=DR,
                                 start=True, stop=False)
                nc.tensor.matmul(ps, ones1[0:1, :128], b_att1r[k][0:1, :],
                                 start=False, stop=True)
                if bblk % 2 == 0:
                    nc.scalar.copy(ckt_t[:, bblk, :], ps)
                else:
                    nc.vector.tensor_copy(ckt_t[:, bblk, :], ps)
            ckT.append(ckt_t)

    # ================= phase B: per-block gram+scan+attention =================
    out_pred = io["out_pred"]
    out_matched = io["out_matched"]
    blk_state = []

    with tc.tile_pool(name="tp", bufs=3) as tp, \
         tc.tile_pool(name="zp", bufs=2) as zp, \
         tc.tile_pool(name="h1p", bufs=4) as h1p, \
         tc.tile_pool(name="h2p", bufs=4) as h2p, \
         tc.tile_pool(name="h3p", bufs=3) as h3p:

        foT_blks = []
        for blk in range(NBLK):
            b0 = blk * PB
            scan_gram = scp.tile([128, 32, 32], F32, tag="scan_gram")
            foT = act.tile([128, 2, PB * NB], FP8, tag=f"foT{blk}")
            foT_blks.append(foT)
            for si in range(NSUB_B):
                s0 = b0 + si * SUB
                fo_nat = tp.tile([128, SUBG, 256], F32, tag="fo_nat")
                ff_nat = tp.tile([128, SUBG, 256], F32, tag="ff_nat")
                nc.sync.dma_start(out=fo_nat, in_=io["f_objects"][s0:s0 + SUB]
                                  .rearrange("(g b) i d -> (b i) g d", b=8))
                nc.sync.dma_start(out=ff_nat, in_=io["f_objects_final"][s0:s0 + SUB]
                                  .rearrange("(g b) i d -> (b i) g d", b=8))
                zt = zp.tile([128, SUB, 2, 32], F32, tag="zt")
                for g in range(SUBG):
                    for c in range(2):
                        ptf_full = ps_tr.tile([128, 512], F32, tag="tr")
                        ptf = ptf_full[:, :128]
                        pe_transpose(nc, ptf, ff_nat[:, g, c * 128:(c + 1) * 128], ident)
                        nc.vector.tensor_copy(zt[:, g * 8:g * 8 + 8, c, 0:16],
                                              ptf.rearrange("p (b j) -> p b j", b=8))
                        pto_full = ps_tr.tile([128, 512], F32, tag="tr")
                        pto = pto_full[:, :128]
                        pe_transpose(nc, pto, fo_nat[:, g, c * 128:(c + 1) * 128], ident)
                        nc.vector.tensor_copy(zt[:, g * 8:g * 8 + 8, c, 16:32],
                                              pto.rearrange("p (b j) -> p b j", b=8))
                        col = (si * SUB + g * 8) * NB
                        nc.scalar.copy(foT[:, c, col:col + 128], pto)
                # gram: 16 batches per psum bank; batch bb -> partitions (bb%4)*32,
                # free cols (bb//4)*32
                gram_sb = sm.tile([128, 2, 128], F32, tag="gram_sb")
                for half in range(2):
                    gp_full = ps_sm.tile([128, 512], F32, tag="sm")
                    gp = gp_full[:, :128]
                    for bb_ in range(16):
                        bb = half * 16 + bb_
                        po = (bb_ % 4) * 32
                        fo_ = (bb_ // 4) * 32
                        for c in range(2):
                            nc.tensor.matmul(gp[po:po + 32, fo_:fo_ + 32],
                                             zt[:, bb, c, :], zt[:, bb, c, :],
                                             start=(c == 0), stop=(c == 1),
                                             tile_position=(0, po))
                    if half == 0:
                        nc.scalar.copy(gram_sb[:, half, :], gp)
                    else:
                        nc.vector.tensor_copy(gram_sb[:, half, :], gp)
                for q in range(SUB // 4):
                    p0 = si * SUB + q * 4
                    eng = nc.sync if q % 2 == 0 else nc.scalar
                    eng.dma_start(out=scan_gram[p0:p0 + 4],
                                  in_=gram_sb[:, q // 4, (q % 4) * 32:(q % 4) * 32 + 32])

            # ---------- matching scan for this block (DVE, transposed base) ----------
            gum = scp.tile([128, NB, NB], F32, tag="gum")
            nc.sync.dma_start(out=gum[:PB], in_=io["gumbel"][b0:b0 + PB])
            bfx = scp.tile([128, NB, 5], F32, tag="bfx")
            nc.sync.dma_start(out=bfx[:PB], in_=io["bboxes_f"][b0:b0 + PB])
            bix = act.tile([128, NB, 5], F32, tag=f"bix{blk}")
            nc.sync.dma_start(out=bix[:PB], in_=io["bboxes_i"][b0:b0 + PB])

            gflat = scan_gram.rearrange("p a b -> p (a b)")
            sq_ff = sm.tile([128, NB], F32, tag="sq_ff")
            nc.scalar.activation(sq_ff[:PB], ap_view(gflat, [list(gflat.ap[0]), [33, 16]])[:PB],
                                 AF.Sqrt)
            inv_ff = sm.tile([128, NB], F32, tag="inv_ff")
            nc.vector.reciprocal(inv_ff[:PB], sq_ff[:PB])
            sq_fo = sm.tile([128, NB], F32, tag="sq_fo")
            nc.scalar.activation(sq_fo[:PB], ap_view(gflat, [list(gflat.ap[0]), [33, 16]], 528)[:PB],
                                 AF.Sqrt)
            inv_fo = sm.tile([128, NB], F32, tag="inv_fo")
            nc.vector.reciprocal(inv_fo[:PB], sq_fo[:PB])

            # baseT[p, i, j] = rho[i,j] + gumbel[i,j]  (rho from gram block [16:32, 0:16])
            baseT = scp.tile([128, NB, NB], F32, tag="baseT")
            nc.vector.tensor_tensor(out=baseT[:PB], in0=scan_gram[:PB, 16:32, 0:16],
                                    in1=inv_fo[:PB].unsqueeze(2).broadcast_to([PB, 16, 16]),
                                    op=ALU.mult)
            nc.vector.tensor_tensor(out=baseT[:PB], in0=baseT[:PB],
                                    in1=inv_ff[:PB].unsqueeze(1).broadcast_to([PB, 16, 16]),
                                    op=ALU.mult)
            nc.vector.tensor_tensor(out=baseT[:PB], in0=baseT[:PB], in1=gum[:PB], op=ALU.add)

            pen = sm.tile([128, NB], F32, tag="pen")
            nc.vector.memset(pen[:PB], 0.0)
            pmat = scp.tile([128, NB, NB], F32, tag="pmat")
            for i in range(NB):
                score = sm.tile([128, NB], F32, tag="score")
                nc.vector.tensor_tensor(out=score[:PB], in0=baseT[:PB, i, :], in1=pen[:PB],
                                        op=ALU.add)
                mx8 = sm.tile([128, 8], F32, tag="mx8")
                nc.vector.max(mx8[:PB], score[:PB])
                oh = sm.tile([128, NB], F32, tag="oh")
                nc.vector.tensor_tensor(out=oh[:PB], in0=score[:PB],
                                        in1=mx8[:PB, 0:1].broadcast_to([PB, NB]),
                                        op=ALU.is_equal)
                nc.vector.scalar_tensor_tensor(out=pen[:PB], in0=oh[:PB], scalar=-1e5,
                                               in1=pen[:PB], op0=ALU.mult, op1=ALU.add)
                nc.vector.tensor_copy(pmat[:PB, i, :], oh[:PB])
            boxes = scp.tile([128, NB, 5], F32, tag="boxes")
            for c_ in range(5):
                prod = sm.tile([128, NB, NB], F32, tag="bprod")
                nc.vector.tensor_tensor(out=prod[:PB], in0=pmat[:PB],
                                        in1=bfx[:PB, :, c_].unsqueeze(1)
                                        .broadcast_to([PB, NB, NB]), op=ALU.mult)
                nc.vector.reduce_sum(boxes[:PB, :, c_], prod[:PB], axis=AX.X)
            nc.sync.dma_start(out=out_matched[b0:b0 + PB], in_=boxes[:PB])

            blk_state.append(bix)

        for blk in range(NBLK):
            b0 = blk * PB
            foT = foT_blks[blk]
            bix = blk_state[blk]
            # ---------- attention for this block ----------
            s4all = act.tile([128, NRT_B, RT], F32, tag=f"s4all{blk}")
            for rt in range(NRT_B):
                col0 = rt * RT
                ps4_full = ps_sm.tile([128, 512], F32, tag="sm")
                ps4 = ps4_full[:, :RT]
                for k in range(4):
                    # --- att1: obj part (fp8 DR) + c_k via mask matmul ---
                    ps1_full = ps_mm.tile([128, 1024], F32, tag="mm")
                    h1 = h1p.tile([128, 2, RT], FP8, tag="h1")
                    for m in range(2):
                        ps = ps1_full[:, m * RT:(m + 1) * RT]
                        nc.tensor.matmul(ps, w_att1[k][:, 2:4, m * 128:(m + 1) * 128],
                                         foT[:, 0:2, col0:col0 + RT], perf_mode=DR,
                                         start=True, stop=False)
                        nc.tensor.matmul(ps, ckT[k][rt * 32:(rt + 1) * 32, blk, m * 128:(m + 1) * 128],
                                         mask128[rt * 32:(rt + 1) * 32].rearrange("p a b -> p (a b)"),
                                         start=False, stop=True,
                                         tile_position=(rt * 32, 0))
                    nc.scalar.activation(h1.rearrange("p c n -> p (c n)"), ps1_full,
                                         AF.Relu)
                    # --- att2 (fp8 DR) ---
                    ps2_full = ps_mm.tile([128, 1024], F32, tag="mm")
                    h2 = h2p.tile([128, 2, RT], FP8, tag="h2")
                    for m in range(2):
                        ps = ps2_full[:, m * RT:(m + 1) * RT]
                        nc.tensor.matmul(ps, w_att2[k][:, 0:2, m * 128:(m + 1) * 128],
                                         h1[:, 0:2, :], perf_mode=DR, start=True, stop=True)
                        if m == 0:
                            nc.scalar.activation(h2[:, m, :], ps, AF.Relu,
                                                 bias=b_att2[k][:, m:m + 1])
                        else:
                            nc.vector.tensor_scalar(out=h2[:, m, :], in0=ps,
                                                    scalar1=b_att2[k][:, m:m + 1],
                                                    scalar2=0.0, op0=ALU.add, op1=ALU.max)
                    # --- att3 (fp8 DR) ---
                    ps3_full = ps_tr.tile([128, 512], F32, tag="tr")
                    ps3 = ps3_full[:, :RT]
                    nc.tensor.matmul(ps3, w_att3[k][:, 0:2, 0:128], h2[:, 0:2, :],
                                     perf_mode=DR, start=True, stop=True)
                    h3 = h3p.tile([128, RT], FP8, tag="h3")
                    nc.scalar.activation(h3, ps3, AF.Relu, bias=b_att3[k][:, 0:1])
                    # --- att4 -> scores at psum partition 32k ---
                    nc.tensor.matmul(ps4[32 * k:32 * k + 1, :], w_att4[k][:, 0, 0:1],
                                     h3, start=True, stop=True,
                                     tile_position=(0, 32 * k))
                nc.vector.tensor_copy(s4all[:, rt, :], ps4)
            sTk = []
            for k in range(4):
                t = sm.tile([128, NB], F32, tag=f"sTk{k}")
                eng = nc.sync if k % 2 == 0 else nc.scalar
                eng.dma_start(out=t[:PB], in_=s4all[32 * k:32 * k + 1, :, :])
                sTk.append(t)

            # ---------- softmax over objects, locs, emb rows ----------
            def softmax_pair(ka, kb, tag):
                z = sm.tile([128, NB], F32, tag=f"z{tag}")
                nc.vector.tensor_tensor(out=z[:PB], in0=sTk[ka][:PB], in1=sTk[kb][:PB],
                                        op=ALU.add)
                e = sm.tile([128, NB], F32, tag=f"e{tag}")
                nc.scalar.activation(e[:PB], z[:PB], AF.Exp)
                ssum = sm.tile([128, 1], F32, tag=f"ss{tag}")
                nc.vector.reduce_sum(ssum[:PB], e[:PB], axis=AX.X)
                rinv = sm.tile([128, 1], F32, tag=f"ri{tag}")
                nc.vector.reciprocal(rinv[:PB], ssum[:PB])
                a = act.tile([128, NB], F32, tag=f"a{tag}{blk}")
                nc.vector.tensor_scalar(out=a[:PB], in0=e[:PB], scalar1=rinv[:PB, 0:1],
                                        scalar2=None, op0=ALU.mult)
                return a

            a_sub = softmax_pair(0, 1, "s")
            a_obj = softmax_pair(2, 3, "o")

            def weighted_loc(a, tag):
                prod = sm.tile([128, 5, NB], F32, tag=f"lp{tag}")
                nc.vector.tensor_tensor(out=prod[:PB], in0=bix[:PB].transpose([0, 2, 1]),
                                        in1=a[:PB].unsqueeze(1).broadcast_to([PB, 5, NB]),
                                        op=ALU.mult)
                loc = sm.tile([128, 5], F32, tag=f"loc{tag}")
                nc.vector.reduce_sum(loc[:PB], prod[:PB], axis=AX.X)
                return loc

            sloc = weighted_loc(a_sub, "s")
            oloc = weighted_loc(a_obj, "o")
            for loc, r0 in ((oloc, 3), (sloc, 8)):
                pt_full = ps_tr.tile([128, 512], F32, tag="tr")
                pt = pt_full[:, :128]
                pe_transpose(nc, pt[:5, :PB], loc[:PB], ident[:PB, :PB])
                locT = sm.tile([5, 128], FP8, tag="locT")
                nc.scalar.copy(locT[:, :PB], pt[:5, :PB])
                nc.sync.dma_start(out=emb[r0:r0 + 5, b0:b0 + PB], in_=locT[:, :PB])
            blk_state[blk] = (a_sub, bix)

        # ================= phase C: pred MLP + blend =================
        p1 = hp_big.tile([128, 2, BL], FP8, tag="pred")
        for m in range(2):
            ps_full = ps_mm.tile([128, 1024], F32, tag="mm")
            ps = ps_full[:, :BL]
            nc.tensor.matmul(ps, w_pred[0][:13, 0, m * 128:(m + 1) * 128], emb[:13],
                             start=True, stop=True)
            eng = nc.scalar if m == 0 else nc.vector
            if eng is nc.scalar:
                nc.scalar.activation(p1[:, m, :], ps, AF.Relu, bias=b_pred[0][:, m:m + 1])
            else:
                nc.vector.tensor_scalar(out=p1[:, m, :], in0=ps,
                                        scalar1=b_pred[0][:, m:m + 1], scalar2=0.0,
                                        op0=ALU.add, op1=ALU.max)
        h = p1
        for li, (K, M) in enumerate([(256, 512), (512, 512), (512, 256)], 1):
            h = mlp_layer(hp_big, h, w_pred[li], b_pred[li], K // 128, M, tag="pred")
        ps5_full = ps_sm.tile([128, 512], F32, tag="sm")
        ps5 = ps5_full[:, :BL]
        for c in range(2):
            nc.tensor.matmul(ps5[:5], w_pred[4][:, c, 0:5], h[:, c, :],
                             start=(c == 0), stop=(c == 1))
        predv = act.tile([5, BL], F32)
        nc.scalar.activation(predv, ps5[:5], AF.Tanh, bias=b_pred5[:5, 0:1])

        for blk in range(NBLK):
            b0 = blk * PB
            a_sub, bix = blk_state[blk]
            pt_full = ps_tr.tile([128, 512], F32, tag="tr")
            pt = pt_full[:, :128]
            pe_transpose(nc, pt[:PB, :5], predv[:, b0:b0 + PB], ident[:5, :5])
            predT = sm.tile([128, 5], F32, tag="predT")
            nc.vector.tensor_copy(predT[:PB], pt[:PB, :5])
            d = scp.tile([128, NB, 5], F32, tag="d")
            nc.vector.tensor_tensor(out=d[:PB], in0=predT[:PB].unsqueeze(1)
                                    .broadcast_to([PB, NB, 5]), in1=bix[:PB], op=ALU.subtract)
            nc.vector.tensor_tensor(out=d[:PB], in0=d[:PB],
                                    in1=a_sub[:PB].unsqueeze(2).broadcast_to([PB, NB, 5]),
                                    op=ALU.mult)
            outb = scp.tile([128, NB, 5], F32, tag="outb")
            nc.vector.tensor_tensor(out=outb[:PB], in0=d[:PB], in1=bix[:PB], op=ALU.add)
            nc.sync.dma_start(out=out_pred[b0:b0 + PB], in_=outb[:PB])

    ctx.close()


INPUT_SPECS = [
    ("f_objects", (NB, DV)), ("f_objects_final", (NB, DV)),
    ("bboxes_i", (NB, 5)), ("bboxes_f", (NB, 5)),
    ("f_instruction", (DL,)), ("gumbel", (NB, NB)),
]
WEIGHT_SPECS = (
    [(f"dec_W{i}", s) for i, s in enumerate([(1024, 512), (512, 256), (256, 256), (256, 128), (128, 3)], 1)]
    + [(f"dec_b{i}", (s,)) for i, s in enumerate([512, 256, 256, 128, 3], 1)]
    + [("map_W1", (4, 1024, 512)), ("map_b1", (4, 512)), ("map_W2", (4, 512, 256)), ("map_b2", (4, 256))]
    + [(f"att_W{i}", (4,) + s) for i, s in enumerate([(512, 256), (256, 256), (256, 128), (128, 1)], 1)]
    + [(f"att_b{i}", (4, s)) for i, s in enumerate([256, 256, 128, 1], 1)]
    + [(f"pred_W{i}", s) for i, s in enumerate([(13, 256), (256, 512), (512, 512), (512, 256), (256, 5)], 1)]
    + [(f"pred_b{i}", (s,)) for i, s in enumerate([256, 512, 512, 256, 5], 1)]
)


def declare_io(nc, BL):
    io = {}
    for name, tail in INPUT_SPECS:
        io[name] = nc.dram_tensor(name, [BL] + list(tail), F32, kind="ExternalInput").ap()
    for name, shape in WEIGHT_SPECS:
        io[name] = nc.dram_tensor(name, list(shape), F32, kind="ExternalInput").ap()
    io["out_pred"] = nc.dram_tensor("out_pred", [BL, NB, 5], F32, kind="ExternalOutput").ap()
    io["out_matched"] = nc.dram_tensor("out_matched", [BL, NB, 5], F32, kind="ExternalOutput").ap()
    return io
# ======================= SPMD driver =======================
import numpy as np

N_CORES = 8
B_FULL = 4096
BL_CORE = B_FULL // N_CORES

_BATCH_INPUTS = ("f_objects", "f_objects_final", "bboxes_i", "bboxes_f",
                 "f_instruction", "gumbel")

_NC = None


def _get_nc():
    global _NC
    if _NC is None:
        from concourse import bacc
        import concourse.tile as tile
        nc = bacc.Bacc("TRN2", target_bir_lowering=False, debug=False,
                       num_devices=N_CORES)
        io = declare_io(nc, BL_CORE)
        with tile.TileContext(nc) as tc:
            build_kernel(tc, io, BL_CORE)
        nc.compile()
        _NC = nc
    return _NC


def kernel(**inputs):
    from concourse.bass_utils import run_bass_kernel_spmd
    nc = _get_nc()
    arrs = {k: np.ascontiguousarray(np.asarray(v, dtype=np.float32))
            for k, v in inputs.items()}
    in_maps = []
    for c in range(N_CORES):
        m = {}
        for k, v in arrs.items():
            if k in _BATCH_INPUTS:
                m[k] = v[c * BL_CORE:(c + 1) * BL_CORE]
            else:
                m[k] = v
        in_maps.append(m)
    res = run_bass_kernel_spmd(nc, in_maps, list(range(N_CORES)))
    pred = np.concatenate([res.results[c]["out_pred"] for c in range(N_CORES)], axis=0)
    matched = np.concatenate([res.results[c]["out_matched"] for c in range(N_CORES)], axis=0)
    return pred, matched


# revision 23
# speedup vs baseline: 760.0539x; 1.0011x over previous
"""Bass/Tile kernel for nn_BaselineModel (gumbel matching + attention MLPs).

v2 layout/precision plan:
  - MLPs in [feature-on-partition, row-on-free], fp8e4 operands with DoubleRow
    matmuls (K>=256), f32 psum. Matching stays f32 (argmax-flip safety).
  - Per-batch map-branch contribution c_k enters the att1 PSUM group via a
    K=32 one-hot "mask matmul" (batch -> 16 rows), so h1 is a single ACT
    Relu from PSUM (no gpsimd, no DVE broadcast add).
  - att4 scores for the 4 branches land in one PSUM bank at partitions
    {0,32,64,96} (tile_position col groups); one copy + one DMA per (k,blk)
    delivers contiguous [batch, object] score tiles. b_att4 is dropped
    (softmax shift invariance).
  - Matching scan runs on the transposed rho block (free transpose from the
    symmetric Z Z^T gram), so gumbel needs no transpose and every DVE op in
    the scan is contiguous. Matched boxes = accumulated one-hot matrix
    applied post-loop.
"""
import sys
sys.path.insert(0, "/opt/trn_rl_repo")
from contextlib import ExitStack
import concourse.bass as bass
import concourse.mybir as mybir
from concourse.masks import make_identity

F32 = mybir.dt.float32
BF16 = mybir.dt.bfloat16
FP8 = mybir.dt.float8e4
AF = mybir.ActivationFunctionType
ALU = mybir.AluOpType
AX = mybir.AxisListType
DR = mybir.MatmulPerfMode.DoubleRow

NB = 16    # objects per batch
DV = 256   # visual feature dim
DL = 1024  # instruction dim


def ap_view(ap, dims, extra_offset=0):
    return bass.AP(tensor=ap.tensor, offset=ap.offset + extra_offset, ap=list(dims))


def pe_transpose(nc, out, in_, ident):
    return nc.tensor.matmul(out, in_, ident, is_transpose=True, start=True, stop=True)


def build_kernel(tc, io, BL):
    """io: dict name -> DRAM AP (inputs + out_pred, out_matched). BL: batches/core."""
    nc = tc.nc
    assert BL % 32 == 0
    SUB = 32                    # gram sub-block (batches)
    PB = min(128, BL)           # scan block (batches)
    NBLK = BL // PB
    NSUB_B = PB // SUB          # subs per block
    SUBG = SUB // 8             # groups of 8 batches per sub
    RT = 512                    # rows per attention tile
    BRT = RT // NB              # batches per attention row tile (32)
    NRT_B = PB // BRT           # row tiles per block

    ctx = ExitStack()

    # ---------- persistent pools ----------
    wpool = ctx.enter_context(tc.tile_pool(name="wpool", bufs=1))
    act = ctx.enter_context(tc.tile_pool(name="act", bufs=1))
    hp_big = ctx.enter_context(tc.tile_pool(name="hp_big", bufs=6))
    sm = ctx.enter_context(tc.tile_pool(name="sm", bufs=2))
    scp = ctx.enter_context(tc.tile_pool(name="scp", bufs=2))
    ps_mm = ctx.enter_context(tc.tile_pool(name="ps_mm", bufs=2, space="PSUM"))
    ps_sm = ctx.enter_context(tc.tile_pool(name="ps_sm", bufs=2, space="PSUM"))
    ps_tr = ctx.enter_context(tc.tile_pool(name="ps_tr", bufs=2, space="PSUM"))

    ident = wpool.tile([128, 128], F32)
    make_identity(nc, ident)
    ones = wpool.tile([16, 16], F32)
    nc.vector.memset(ones, 1.0)
    ones1 = wpool.tile([1, 128], FP8)
    nc.vector.memset(ones1, 1.0)
    # one-hot batch->row expansion: mask128[32g + j, b*NB + n] = (b == j)
    mask128 = wpool.tile([128, 32, NB], BF16)
    for g in range(4):
        nc.vector.tensor_copy(
            mask128[g * 32:(g + 1) * 32],
            ident[g * 32:(g + 1) * 32, g * 32:(g + 1) * 32].unsqueeze(2)
            .broadcast_to([32, 32, NB]))

    def load_bias_col(pool, name, M, k=None):
        """DRAM [M] (or [4,M] row k) -> [p, mb] tile, column m = feats m*128..."""
        mb = (M + 127) // 128
        p = min(M, 128)
        t = pool.tile([128, mb], F32, tag=f"b_{name}{'' if k is None else k}")
        off = 0 if k is None else k * M
        nc.sync.dma_start(out=t[:p, :], in_=ap_view(io[name], [[1, p], [128, mb]], off))
        return t

    def load_bias_row(pool, name, M, k=None):
        """DRAM [M] (or [4,M] row k) -> [1, M] row tile."""
        t = pool.tile([1, M], FP8, tag=f"br_{name}{'' if k is None else k}")
        off = 0 if k is None else k * M
        nc.gpsimd.dma_start(out=t, in_=ap_view(io[name], [[M, 1], [1, M]], off))
        return t

    # ================= phase A: weights for dec/map (scoped) =================
    with tc.tile_pool(name="wdec", bufs=1) as wdec, \
         tc.tile_pool(name="fip", bufs=2) as fip:

        def load_w(pool, name, K, M, k=None, dt=FP8):
            kc = (K + 127) // 128
            p = min(K, 128)
            t = pool.tile([128, kc, M], dt, tag=f"w_{name}{'' if k is None else k}")
            src = io[name] if k is None else io[name][k]
            view = src.rearrange("(c p) m -> p c m", p=128) if K >= 128 else src.unsqueeze(1)
            nc.gpsimd.dma_start(out=t[:p], in_=view)
            return t

        # ---------- f_instruction -> finstT fp8 [128, 8, BL] ----------
        finstT = act.tile([128, 8, BL], FP8)
        nbt = max(1, BL // 128)
        btr = min(BL, 128)
        for bt in range(nbt):
            fi = fip.tile([128, DL], F32, tag="fi")
            nc.sync.dma_start(out=fi[:btr], in_=io["f_instruction"][bt * 128:bt * 128 + btr])
            for kc in range(8):
                pt_full = ps_tr.tile([128, 512], F32, tag="tr")
                pt = pt_full[:, :128]
                pe_transpose(nc, pt[:, :btr], fi[:btr, kc * 128:(kc + 1) * 128], ident[:btr, :btr])
                if kc % 2 == 0:
                    nc.scalar.copy(finstT[:, kc, bt * 128:bt * 128 + btr], pt[:, :btr])
                else:
                    nc.vector.tensor_copy(finstT[:, kc, bt * 128:bt * 128 + btr], pt[:, :btr])

        w_dec = [load_w(wdec, f"dec_W{i}", K, M) for i, (K, M) in
                 enumerate([(1024, 512), (512, 256), (256, 256), (256, 128), (128, 3)], 1)]
        b_dec = [load_bias_col(wdec, f"dec_b{i}", M) for i, M in
                 enumerate([512, 256, 256, 128], 1)]
        b_dec5 = load_bias_col(wpool, "dec_b5", 3)
        w_map1 = [load_w(wdec, "map_W1", 1024, 512, k) for k in range(4)]
        w_map2 = [load_w(wdec, "map_W2", 512, 256, k) for k in range(4)]
        b_map1 = [load_bias_col(wdec, "map_b1", 512, k) for k in range(4)]
        b_map2 = [load_bias_col(wdec, "map_b2", 256, k) for k in range(4)]
        # att/pred weights persist
        w_att1 = [load_w(wpool, "att_W1", 512, 256, k) for k in range(4)]
        w_att2 = [load_w(wpool, "att_W2", 256, 256, k) for k in range(4)]
        w_att3 = [load_w(wpool, "att_W3", 256, 128, k) for k in range(4)]
        w_att4 = [load_w(wpool, "att_W4", 128, 1, k) for k in range(4)]
        b_att1r = [load_bias_row(wpool, "att_b1", 256, k) for k in range(4)]
        b_att2 = [load_bias_col(wpool, "att_b2", 256, k) for k in range(4)]
        b_att3 = [load_bias_col(wpool, "att_b3", 128, k) for k in range(4)]
        w_pred = [load_w(wpool, f"pred_W{i}", K, M) for i, (K, M) in
                  enumerate([(13, 256), (256, 512), (512, 512), (512, 256), (256, 5)], 1)]
        b_pred = [load_bias_col(wpool, f"pred_b{i}", M) for i, M in
                  enumerate([256, 512, 512, 256], 1)]
        b_pred5 = load_bias_col(wpool, "pred_b5", 5)

        def mm_dr(ps, w, x, kc, m, mw, n_lo, n_hi):
            """psum[:mw, :] += sum_c w[:,c,m128:+mw].T @ x[:,c,n_lo:n_hi], DoubleRow pairs."""
            kc2 = kc // 2
            for c2 in range(kc2):
                nc.tensor.matmul(ps[:mw], w[:, 2 * c2:2 * c2 + 2, m * 128:m * 128 + mw],
                                 x[:, 2 * c2:2 * c2 + 2, n_lo:n_hi], perf_mode=DR,
                                 start=(c2 == 0), stop=(c2 == kc2 - 1))

        def mlp_layer(pool, x, w, b, kc, M, act_fn=AF.Relu, tag="h"):
            """x: [128, kc, BL] fp8 -> out [128, M/128, BL] fp8 via DR matmuls + ACT."""
            mb = (M + 127) // 128
            o = pool.tile([128, mb, BL], FP8, tag=tag)
            for m in range(mb):
                mw = min(128, M - m * 128)
                ps_full = ps_mm.tile([128, 1024], F32, tag="mm")
                ps = ps_full[:, :BL]
                mm_dr(ps, w, x, kc, m, mw, 0, BL)
                if m % 2 == 0:
                    nc.scalar.activation(o[:mw, m, :], ps[:mw], act_fn, bias=b[:mw, m:m + 1])
                elif act_fn == AF.Relu:
                    nc.vector.tensor_scalar(out=o[:mw, m, :], in0=ps[:mw],
                                            scalar1=b[:mw, m:m + 1], scalar2=0.0,
                                            op0=ALU.add, op1=ALU.max)
                else:
                    nc.vector.tensor_scalar(out=o[:mw, m, :], in0=ps[:mw],
                                            scalar1=b[:mw, m:m + 1], scalar2=None,
                                            op0=ALU.add)
            return o

        # ---------- dec MLP -> f_action -> emb[0:3] ----------
        emb = act.tile([16, BL], FP8)
        h = finstT
        for li, (K, M) in enumerate([(1024, 512), (512, 256), (256, 256), (256, 128)]):
            h = mlp_layer(hp_big, h, w_dec[li], b_dec[li], K // 128, M, tag="dec")
        ps5_full = ps_mm.tile([128, 1024], F32, tag="mm")
        ps5 = ps5_full[:, :BL]
        nc.tensor.matmul(ps5[:3], w_dec[4][:, 0, 0:3], h[:, 0, :], start=True, stop=True)
        e_sb = sm.tile([3, BL], F32, tag="e_sb")
        nc.scalar.activation(e_sb, ps5[:3], AF.Exp, bias=b_dec5[:3, 0:1])
        ps_s_full = ps_sm.tile([128, 512], F32, tag="sm")
        ps_s = ps_s_full[:, :BL]
        nc.tensor.matmul(ps_s[:1], ones[:3, 0:1], e_sb[:], start=True, stop=True)
        r_sb = sm.tile([1, BL], F32, tag="r_sb")
        nc.vector.reciprocal(r_sb, ps_s[:1])
        ps_rb_full = ps_sm.tile([128, 512], F32, tag="sm")
        ps_rb = ps_rb_full[:, :BL]
        nc.tensor.matmul(ps_rb[:3], ones[0:1, 0:3], r_sb[:], start=True, stop=True)
        nc.vector.tensor_tensor(out=emb[0:3], in0=e_sb[:], in1=ps_rb[:3], op=ALU.mult)

        # ---------- map MLPs -> c_kT [batch, feat] bf16 (includes b_att1) ----------
        ckT = []
        for k in range(4):
            h1m = mlp_layer(hp_big, finstT, w_map1[k], b_map1[k], 8, 512, tag="map1")
            mk = mlp_layer(hp_big, h1m, w_map2[k], b_map2[k], 4, 256,
                           act_fn=AF.Identity, tag="mk")
            # ckt_t: [128 batch-in-block, nblk, 256 feat], includes b_att1
            ckt_t = act.tile([128, BL // 128, 256], BF16, tag=f"ckT{k}")
            for bblk in range(BL // 128):
                ps_full = ps_sm.tile([128, 512], F32, tag="sm")
                ps = ps_full[:, :256]
                nc.tensor.matmul(ps, mk[:, 0:2, bblk * 128:bblk * 128 + 128],
                                 w_att1[k][:, 0:2, 0:256], perf_mode=DR,
                                 start=True, stop=False)
                nc.tensor.matmul(ps, ones1[0:1, :128], b_att1r[k][0:1, :],
                                 start=False, stop=True)
                if bblk % 2 == 0:
                    nc.scalar.copy(ckt_t[:, bblk, :], ps)
                else:
                    nc.vector.tensor_copy(ckt_t[:, bblk, :], ps)
            ckT.append(ckt_t)

    # ================= phase B: per-block gram+scan+attention =================
    out_pred = io["out_pred"]
    out_matched = io["out_matched"]
    blk_state = []

    with tc.tile_pool(name="tp", bufs=3) as tp, \
         tc.tile_pool(name="zp", bufs=2) as zp, \
         tc.tile_pool(name="h1p", bufs=4) as h1p, \
         tc.tile_pool(name="h2p", bufs=4) as h2p, \
         tc.tile_pool(name="h3p", bufs=3) as h3p:

        foT_blks = []
        blk_tiles = {}

        def gram_pre(blk):
            scan_gram = scp.tile([128, 32, 32], F32, tag="scan_gram")
            foT = act.tile([128, 2, PB * NB], FP8, tag=f"foT{blk}")
            foT_blks.append(foT)
            blk_tiles[blk] = (scan_gram, foT)

        def gram_sub(blk, si):
            scan_gram, foT = blk_tiles[blk]
            s0 = blk * PB + si * SUB
            fo_nat = tp.tile([128, SUBG, 256], F32, tag="fo_nat")
            ff_nat = tp.tile([128, SUBG, 256], F32, tag="ff_nat")
            nc.sync.dma_start(out=fo_nat, in_=io["f_objects"][s0:s0 + SUB]
                              .rearrange("(g b) i d -> (b i) g d", b=8))
            nc.sync.dma_start(out=ff_nat, in_=io["f_objects_final"][s0:s0 + SUB]
                              .rearrange("(g b) i d -> (b i) g d", b=8))
            zt = zp.tile([128, SUB, 2, 32], F32, tag="zt")
            for g in range(SUBG):
                for c in range(2):
                    # both transposes share one psum bank: ff -> cols 0:128, fo -> 128:256
                    pt_full = ps_tr.tile([128, 512], F32, tag="tr")
                    pe_transpose(nc, pt_full[:, 0:128],
                                 ff_nat[:, g, c * 128:(c + 1) * 128], ident)
                    pe_transpose(nc, pt_full[:, 128:256],
                                 fo_nat[:, g, c * 128:(c + 1) * 128], ident)
                    # one copy fills zt[:, 8b, c, 0:16 (ff) | 16:32 (fo)]
                    nc.vector.tensor_copy(
                        zt[:, g * 8:g * 8 + 8, c, :],
                        ap_view(pt_full, [list(pt_full.ap[0]), [16, 8], [128, 2], [1, 16]]))
                    col = (si * SUB + g * 8) * NB
                    nc.scalar.copy(foT[:, c, col:col + 128], pt_full[:, 128:256])
            # gram: 16 batches per psum bank; batch bb -> partitions (bb%4)*32,
            # free cols (bb//4)*32
            gram_sb = sm.tile([128, 2, 128], F32, tag="gram_sb")
            for half in range(2):
                gp_full = ps_sm.tile([128, 512], F32, tag="sm")
                gp = gp_full[:, :128]
                for bb_ in range(16):
                    bb = half * 16 + bb_
                    po = (bb_ % 4) * 32
                    fo_ = (bb_ // 4) * 32
                    for c in range(2):
                        nc.tensor.matmul(gp[po:po + 32, fo_:fo_ + 32],
                                         zt[:, bb, c, :], zt[:, bb, c, :],
                                         start=(c == 0), stop=(c == 1),
                                         tile_position=(0, po))
                if half == 0:
                    nc.scalar.copy(gram_sb[:, half, :], gp)
                else:
                    nc.vector.tensor_copy(gram_sb[:, half, :], gp)
            for q in range(SUB // 4):
                p0 = si * SUB + q * 4
                eng = nc.sync if q % 2 == 0 else nc.scalar
                eng.dma_start(out=scan_gram[p0:p0 + 4],
                              in_=gram_sb[:, q // 4, (q % 4) * 32:(q % 4) * 32 + 32])

        def scan_blk(blk):
            # ---------- matching scan for this block (DVE, transposed base) ----------
            scan_gram, _ = blk_tiles[blk]
            b0 = blk * PB
            gum = scp.tile([128, NB, NB], F32, tag="gum")
            nc.sync.dma_start(out=gum[:PB], in_=io["gumbel"][b0:b0 + PB])
            bfx = scp.tile([128, NB, 5], F32, tag="bfx")
            nc.sync.dma_start(out=bfx[:PB], in_=io["bboxes_f"][b0:b0 + PB])
            bix = act.tile([128, NB, 5], F32, tag=f"bix{blk}")
            nc.sync.dma_start(out=bix[:PB], in_=io["bboxes_i"][b0:b0 + PB])

            gflat = scan_gram.rearrange("p a b -> p (a b)")
            sq_ff = sm.tile([128, NB], F32, tag="sq_ff")
            nc.scalar.activation(sq_ff[:PB], ap_view(gflat, [list(gflat.ap[0]), [33, 16]])[:PB],
                                 AF.Sqrt)
            inv_ff = sm.tile([128, NB], F32, tag="inv_ff")
            nc.vector.reciprocal(inv_ff[:PB], sq_ff[:PB])
            sq_fo = sm.tile([128, NB], F32, tag="sq_fo")
            nc.scalar.activation(sq_fo[:PB], ap_view(gflat, [list(gflat.ap[0]), [33, 16]], 528)[:PB],
                                 AF.Sqrt)
            inv_fo = sm.tile([128, NB], F32, tag="inv_fo")
            nc.vector.reciprocal(inv_fo[:PB], sq_fo[:PB])

            # baseT[p, i, j] = rho[i,j] + gumbel[i,j]  (rho from gram block [16:32, 0:16])
            baseT = scp.tile([128, NB, NB], F32, tag="baseT")
            nc.vector.tensor_tensor(out=baseT[:PB], in0=scan_gram[:PB, 16:32, 0:16],
                                    in1=inv_fo[:PB].unsqueeze(2).broadcast_to([PB, 16, 16]),
                                    op=ALU.mult)
            nc.vector.tensor_tensor(out=baseT[:PB], in0=baseT[:PB],
                                    in1=inv_ff[:PB].unsqueeze(1).broadcast_to([PB, 16, 16]),
                                    op=ALU.mult)
            nc.vector.tensor_tensor(out=baseT[:PB], in0=baseT[:PB], in1=gum[:PB], op=ALU.add)

            pen = sm.tile([128, NB], F32, tag="pen")
            nc.vector.memset(pen[:PB], 0.0)
            pmat = scp.tile([128, NB, NB], F32, tag="pmat")
            for i in range(NB):
                score = sm.tile([128, NB], F32, tag="score")
                nc.vector.tensor_tensor(out=score[:PB], in0=baseT[:PB, i, :], in1=pen[:PB],
                                        op=ALU.add)
                mx8 = sm.tile([128, 8], F32, tag="mx8")
                nc.vector.max(mx8[:PB], score[:PB])
                oh = sm.tile([128, NB], F32, tag="oh")
                nc.vector.tensor_tensor(out=oh[:PB], in0=score[:PB],
                                        in1=mx8[:PB, 0:1].broadcast_to([PB, NB]),
                                        op=ALU.is_equal)
                nc.vector.scalar_tensor_tensor(out=pen[:PB], in0=oh[:PB], scalar=-1e5,
                                               in1=pen[:PB], op0=ALU.mult, op1=ALU.add)
                nc.vector.tensor_copy(pmat[:PB, i, :], oh[:PB])
            boxes = scp.tile([128, NB, 5], F32, tag="boxes")
            for c_ in range(5):
                prod = sm.tile([128, NB, NB], F32, tag="bprod")
                nc.vector.tensor_tensor(out=prod[:PB], in0=pmat[:PB],
                                        in1=bfx[:PB, :, c_].unsqueeze(1)
                                        .broadcast_to([PB, NB, NB]), op=ALU.mult)
                nc.vector.reduce_sum(boxes[:PB, :, c_], prod[:PB], axis=AX.X)
            nc.sync.dma_start(out=out_matched[b0:b0 + PB], in_=boxes[:PB])

            blk_state.append(bix)

        att_s4 = {}

        def att_rt(blk, rt):
            foT = foT_blks[blk]
            if rt == 0:
                s4all = act.tile([128, NRT_B, RT], F32, tag=f"s4all{blk}")
                att_s4[blk] = s4all
            s4all = att_s4[blk]
            col0 = rt * RT
            ps4_full = ps_sm.tile([128, 512], F32, tag="sm")
            ps4 = ps4_full[:, :RT]
            for k in range(4):
                # --- att1: obj part (fp8 DR) + c_k via mask matmul ---
                ps1_full = ps_mm.tile([128, 1024], F32, tag="mm")
                h1 = h1p.tile([128, 2, RT], FP8, tag="h1")
                for m in range(2):
                    ps = ps1_full[:, m * RT:(m + 1) * RT]
                    nc.tensor.matmul(ps, w_att1[k][:, 2:4, m * 128:(m + 1) * 128],
                                     foT[:, 0:2, col0:col0 + RT], perf_mode=DR,
                                     start=True, stop=False)
                    nc.tensor.matmul(ps, ckT[k][rt * 32:(rt + 1) * 32, blk, m * 128:(m + 1) * 128],
                                     mask128[rt * 32:(rt + 1) * 32].rearrange("p a b -> p (a b)"),
                                     start=False, stop=True,
                                     tile_position=(rt * 32, 0))
                    if m == 0:
                        nc.scalar.activation(h1[:, m, :], ps, AF.Relu)
                    else:
                        nc.vector.tensor_scalar(out=h1[:, m, :], in0=ps,
                                                scalar1=0.0, scalar2=None, op0=ALU.max)
                # --- att2 (fp8 DR) ---
                ps2_full = ps_mm.tile([128, 1024], F32, tag="mm")
                h2 = h2p.tile([128, 2, RT], FP8, tag="h2")
                for m in range(2):
                    ps = ps2_full[:, m * RT:(m + 1) * RT]
                    nc.tensor.matmul(ps, w_att2[k][:, 0:2, m * 128:(m + 1) * 128],
                                     h1[:, 0:2, :], perf_mode=DR, start=True, stop=True)
                    if m == 0:
                        nc.scalar.activation(h2[:, m, :], ps, AF.Relu,
                                             bias=b_att2[k][:, m:m + 1])
                    else:
                        nc.vector.tensor_scalar(out=h2[:, m, :], in0=ps,
                                                scalar1=b_att2[k][:, m:m + 1],
                                                scalar2=0.0, op0=ALU.add, op1=ALU.max)
                # --- att3 (fp8 DR) ---
                ps3_full = ps_mm.tile([128, 1024], F32, tag="mm")
                ps3 = ps3_full[:, :RT]
                nc.tensor.matmul(ps3, w_att3[k][:, 0:2, 0:128], h2[:, 0:2, :],
                                 perf_mode=DR, start=True, stop=True)
                h3 = h3p.tile([128, RT], FP8, tag="h3")
                nc.scalar.activation(h3, ps3, AF.Relu, bias=b_att3[k][:, 0:1])
                # --- att4 -> scores at psum partition 32k ---
                nc.tensor.matmul(ps4[32 * k:32 * k + 1, :], w_att4[k][:, 0, 0:1],
                                 h3, start=True, stop=True,
                                 tile_position=(0, 32 * k))
            nc.vector.tensor_copy(s4all[:, rt, :], ps4)

        def att_post(blk):
            b0 = blk * PB
            a_sub, bix = _att_post(blk)
            blk_state[blk] = (a_sub, bix)

        def _att_post(blk):
            b0 = blk * PB
            bix = blk_state[blk]
            s4all = att_s4[blk]
            sTk = []
            for k in range(4):
                t = sm.tile([128, NB], F32, tag=f"sTk{k}")
                eng = nc.sync if k % 2 == 0 else nc.scalar
                eng.dma_start(out=t[:PB], in_=s4all[32 * k:32 * k + 1, :, :])
                sTk.append(t)

            # ---------- softmax over objects, locs, emb rows ----------
            def softmax_pair(ka, kb, tag):
                z = sm.tile([128, NB], F32, tag=f"z{tag}")
                nc.vector.tensor_tensor(out=z[:PB], in0=sTk[ka][:PB], in1=sTk[kb][:PB],
                                        op=ALU.add)
                e = sm.tile([128, NB], F32, tag=f"e{tag}")
                nc.scalar.activation(e[:PB], z[:PB], AF.Exp)
                ssum = sm.tile([128, 1], F32, tag=f"ss{tag}")
                nc.vector.reduce_sum(ssum[:PB], e[:PB], axis=AX.X)
                rinv = sm.tile([128, 1], F32, tag=f"ri{tag}")
                nc.vector.reciprocal(rinv[:PB], ssum[:PB])
                a = act.tile([128, NB], F32, tag=f"a{tag}{blk}")
                nc.vector.tensor_scalar(out=a[:PB], in0=e[:PB], scalar1=rinv[:PB, 0:1],
                                        scalar2=None, op0=ALU.mult)
                return a

            a_sub = softmax_pair(0, 1, "s")
            a_obj = softmax_pair(2, 3, "o")

            def weighted_loc(a, tag):
                prod = sm.tile([128, 5, NB], F32, tag=f"lp{tag}")
                nc.vector.tensor_tensor(out=prod[:PB], in0=bix[:PB].transpose([0, 2, 1]),
                                        in1=a[:PB].unsqueeze(1).broadcast_to([PB, 5, NB]),
                                        op=ALU.mult)
                loc = sm.tile([128, 5], F32, tag=f"loc{tag}")
                nc.vector.reduce_sum(loc[:PB], prod[:PB], axis=AX.X)
                return loc

            sloc = weighted_loc(a_sub, "s")
            oloc = weighted_loc(a_obj, "o")
            b0_ = blk * PB
            for loc, r0 in ((oloc, 3), (sloc, 8)):
                pt_full = ps_tr.tile([128, 512], F32, tag="tr")
                pt = pt_full[:, :128]
                pe_transpose(nc, pt[:5, :PB], loc[:PB], ident[:PB, :PB])
                locT = sm.tile([5, 128], FP8, tag="locT")
                nc.scalar.copy(locT[:, :PB], pt[:5, :PB])
                nc.sync.dma_start(out=emb[r0:r0 + 5, b0_:b0_ + PB], in_=locT[:, :PB])
            return a_sub, bix

        # ---- driver: interleave gram(blk) with attention(blk-1) to keep PE dense ----
        for blk in range(NBLK + 1):
            if blk < NBLK:
                gram_pre(blk)
            gunits = list(range(NSUB_B)) if blk < NBLK else []
            aunits = list(range(NRT_B)) if blk >= 1 else []
            for u in range(max(len(gunits), len(aunits))):
                if u < len(aunits):
                    att_rt(blk - 1, aunits[u])
                if u < len(gunits):
                    gram_sub(blk, gunits[u])
            if blk >= 1:
                att_post(blk - 1)
            if blk < NBLK:
                scan_blk(blk)

        # ================= phase C: pred MLP + blend =================
        p1 = hp_big.tile([128, 2, BL], FP8, tag="pred")
        for m in range(2):
            ps_full = ps_mm.tile([128, 1024], F32, tag="mm")
            ps = ps_full[:, :BL]
            nc.tensor.matmul(ps, w_pred[0][:13, 0, m * 128:(m + 1) * 128], emb[:13],
                             start=True, stop=True)
            eng = nc.scalar if m == 0 else nc.vector
            if eng is nc.scalar:
                nc.scalar.activation(p1[:, m, :], ps, AF.Relu, bias=b_pred[0][:, m:m + 1])
            else:
                nc.vector.tensor_scalar(out=p1[:, m, :], in0=ps,
                                        scalar1=b_pred[0][:, m:m + 1], scalar2=0.0,
                                        op0=ALU.add, op1=ALU.max)
        h = p1
        for li, (K, M) in enumerate([(256, 512), (512, 512), (512, 256)], 1):
            h = mlp_layer(hp_big, h, w_pred[li], b_pred[li], K // 128, M, tag="pred")
        ps5_full = ps_sm.tile([128, 512], F32, tag="sm")
        ps5 = ps5_full[:, :BL]
        for c in range(2):
            nc.tensor.matmul(ps5[:5], w_pred[4][:, c, 0:5], h[:, c, :],
                             start=(c == 0), stop=(c == 1))
        predv = act.tile([5, BL], F32)
        nc.scalar.activation(predv, ps5[:5], AF.Tanh, bias=b_pred5[:5, 0:1])

        for blk in range(NBLK):
            b0 = blk * PB
            a_sub, bix = blk_state[blk]
            pt_full = ps_tr.tile([128, 512], F32, tag="tr")
            pt = pt_full[:, :128]
            pe_transpose(nc, pt[:PB, :5], predv[:, b0:b0 + PB], ident[:5, :5])
            predT = sm.tile([128, 5], F32, tag="predT")
            nc.vector.tensor_copy(predT[:PB], pt[:PB, :5])
            d = scp.tile([128, NB, 5], F32, tag="d")
            nc.vector.tensor_tensor(out=d[:PB], in0=predT[:PB].unsqueeze(1)
                                    .broadcast_to([PB, NB, 5]), in1=bix[:PB], op=ALU.subtract)
            nc.vector.tensor_tensor(out=d[:PB], in0=d[:PB],
                                    in1=a_sub[:PB].unsqueeze(2).broadcast_to([PB, NB, 5]),
                                    op=ALU.mult)
            outb = scp.tile([128, NB, 5], F32, tag="outb")
            nc.vector.tensor_tensor(out=outb[:PB], in0=d[:PB], in1=bix[:PB], op=ALU.add)
            nc.sync.dma_start(out=out_pred[b0:b0 + PB], in_=outb[:PB])

    ctx.close()


INPUT_SPECS = [
    ("f_objects", (NB, DV)), ("f_objects_final", (NB, DV)),
    ("bboxes_i", (NB, 5)), ("bboxes_f", (NB, 5)),
    ("f_instruction", (DL,)), ("gumbel", (NB, NB)),
]
WEIGHT_SPECS = (
    [(f"dec_W{i}", s) for i, s in enumerate([(1024, 512), (512, 256), (256, 256), (256, 128), (128, 3)], 1)]
    + [(f"dec_b{i}", (s,)) for i, s in enumerate([512, 256, 256, 128, 3], 1)]
    + [("map_W1", (4, 1024, 512)), ("map_b1", (4, 512)), ("map_W2", (4, 512, 256)), ("map_b2", (4, 256))]
    + [(f"att_W{i}", (4,) + s) for i, s in enumerate([(512, 256), (256, 256), (256, 128), (128, 1)], 1)]
    + [(f"att_b{i}", (4, s)) for i, s in enumerate([256, 256, 128, 1], 1)]
    + [(f"pred_W{i}", s) for i, s in enumerate([(13, 256), (256, 512), (512, 512), (512, 256), (256, 5)], 1)]
    + [(f"pred_b{i}", (s,)) for i, s in enumerate([256, 512, 512, 256, 5], 1)]
)


def declare_io(nc, BL):
    io = {}
    for name, tail in INPUT_SPECS:
        io[name] = nc.dram_tensor(name, [BL] + list(tail), F32, kind="ExternalInput").ap()
    for name, shape in WEIGHT_SPECS:
        io[name] = nc.dram_tensor(name, list(shape), F32, kind="ExternalInput").ap()
    io["out_pred"] = nc.dram_tensor("out_pred", [BL, NB, 5], F32, kind="ExternalOutput").ap()
    io["out_matched"] = nc.dram_tensor("out_matched", [BL, NB, 5], F32, kind="ExternalOutput").ap()
    return io
# ======================= SPMD driver =======================
import numpy as np

N_CORES = 8
B_FULL = 4096
BL_CORE = B_FULL // N_CORES

_BATCH_INPUTS = ("f_objects", "f_objects_final", "bboxes_i", "bboxes_f",
                 "f_instruction", "gumbel")

_NC = None


def _get_nc():
    global _NC
    if _NC is None:
        from concourse import bacc
        import concourse.tile as tile
        nc = bacc.Bacc("TRN2", target_bir_lowering=False, debug=False,
                       num_devices=N_CORES)
        io = declare_io(nc, BL_CORE)
        with tile.TileContext(nc) as tc:
            build_kernel(tc, io, BL_CORE)
        nc.compile()
        _NC = nc
    return _NC


def kernel(**inputs):
    from concourse.bass_utils import run_bass_kernel_spmd
    nc = _get_nc()
    arrs = {k: np.ascontiguousarray(np.asarray(v, dtype=np.float32))
            for k, v in inputs.items()}
    in_maps = []
    for c in range(N_CORES):
        m = {}
        for k, v in arrs.items():
            if k in _BATCH_INPUTS:
                m[k] = v[c * BL_CORE:(c + 1) * BL_CORE]
            else:
                m[k] = v
        in_maps.append(m)
    res = run_bass_kernel_spmd(nc, in_maps, list(range(N_CORES)))
    pred = np.concatenate([res.results[c]["out_pred"] for c in range(N_CORES)], axis=0)
    matched = np.concatenate([res.results[c]["out_matched"] for c in range(N_CORES)], axis=0)
    return pred, matched


# revision 25
# speedup vs baseline: 874.3152x; 1.1503x over previous
"""Bass/Tile kernel for nn_BaselineModel (gumbel matching + attention MLPs).

v2 layout/precision plan:
  - MLPs in [feature-on-partition, row-on-free], fp8e4 operands with DoubleRow
    matmuls (K>=256), f32 psum. Matching stays f32 (argmax-flip safety).
  - Per-batch map-branch contribution c_k enters the att1 PSUM group via a
    K=32 one-hot "mask matmul" (batch -> 16 rows), so h1 is a single ACT
    Relu from PSUM (no gpsimd, no DVE broadcast add).
  - att4 scores for the 4 branches land in one PSUM bank at partitions
    {0,32,64,96} (tile_position col groups); one copy + one DMA per (k,blk)
    delivers contiguous [batch, object] score tiles. b_att4 is dropped
    (softmax shift invariance).
  - Matching scan runs on the transposed rho block (free transpose from the
    symmetric Z Z^T gram), so gumbel needs no transpose and every DVE op in
    the scan is contiguous. Matched boxes = accumulated one-hot matrix
    applied post-loop.
"""
import sys
sys.path.insert(0, "/opt/trn_rl_repo")
from contextlib import ExitStack
import concourse.bass as bass
import concourse.mybir as mybir
from concourse.masks import make_identity

F32 = mybir.dt.float32
BF16 = mybir.dt.bfloat16
FP8 = mybir.dt.float8e4
AF = mybir.ActivationFunctionType
ALU = mybir.AluOpType
AX = mybir.AxisListType
DR = mybir.MatmulPerfMode.DoubleRow

NB = 16    # objects per batch
DV = 256   # visual feature dim
DL = 1024  # instruction dim


def ap_view(ap, dims, extra_offset=0):
    return bass.AP(tensor=ap.tensor, offset=ap.offset + extra_offset, ap=list(dims))


def pe_transpose(nc, out, in_, ident):
    return nc.tensor.matmul(out, in_, ident, is_transpose=True, start=True, stop=True)


def build_kernel(tc, io, BL):
    """io: dict name -> DRAM AP (inputs + out_pred, out_matched). BL: batches/core.

    Phase order (v4): input/weight DMAs queue first; gram+scan (weight-
    independent) runs under the DMA stream; then dec/map MLPs (dense PE,
    warms the clock); then attention; then pred+blend.
    """
    nc = tc.nc
    assert BL % 32 == 0
    SUB = 32                    # gram sub-block (batches)
    PB = min(128, BL)           # scan block (batches)
    NBLK = BL // PB
    NSUB_B = PB // SUB          # subs per block
    SUBG = SUB // 8             # groups of 8 batches per sub
    RT = 512                    # rows per attention tile
    BRT = RT // NB              # batches per attention row tile (32)
    NRT_B = PB // BRT           # row tiles per block

    ctx = ExitStack()

    # ---------- pools ----------
    wpool = ctx.enter_context(tc.tile_pool(name="wpool", bufs=1))
    act = ctx.enter_context(tc.tile_pool(name="act", bufs=1))
    hp_big = ctx.enter_context(tc.tile_pool(name="hp_big", bufs=2))
    sm = ctx.enter_context(tc.tile_pool(name="sm", bufs=2))
    scp = ctx.enter_context(tc.tile_pool(name="scp", bufs=2))
    tp = ctx.enter_context(tc.tile_pool(name="tp", bufs=2))
    zp = ctx.enter_context(tc.tile_pool(name="zp", bufs=2))
    h1p = ctx.enter_context(tc.tile_pool(name="h1p", bufs=4))
    h2p = ctx.enter_context(tc.tile_pool(name="h2p", bufs=4))
    h3p = ctx.enter_context(tc.tile_pool(name="h3p", bufs=3))
    fip = ctx.enter_context(tc.tile_pool(name="fip", bufs=1))
    wdec = ctx.enter_context(tc.tile_pool(name="wdec", bufs=1))
    ps_mm = ctx.enter_context(tc.tile_pool(name="ps_mm", bufs=2, space="PSUM"))
    ps_sm = ctx.enter_context(tc.tile_pool(name="ps_sm", bufs=2, space="PSUM"))
    ps_tr = ctx.enter_context(tc.tile_pool(name="ps_tr", bufs=2, space="PSUM"))

    ident = wpool.tile([128, 128], F32)
    make_identity(nc, ident)
    ones = wpool.tile([16, 16], F32)
    nc.vector.memset(ones, 1.0)
    ones1 = wpool.tile([1, 128], FP8)
    nc.vector.memset(ones1, 1.0)
    # one-hot batch->row expansion: mask128[32g + j, b*NB + n] = (b == j)
    mask128 = wpool.tile([128, 32, NB], BF16)
    for g in range(4):
        nc.vector.tensor_copy(
            mask128[g * 32:(g + 1) * 32],
            ident[g * 32:(g + 1) * 32, g * 32:(g + 1) * 32].unsqueeze(2)
            .broadcast_to([32, 32, NB]))

    def load_bias_col(pool, name, M, k=None):
        """DRAM [M] (or [4,M] row k) -> [p, mb] tile, column m = feats m*128..."""
        mb = (M + 127) // 128
        p = min(M, 128)
        t = pool.tile([128, mb], F32, tag=f"b_{name}{'' if k is None else k}")
        off = 0 if k is None else k * M
        nc.scalar.dma_start(out=t[:p, :], in_=ap_view(io[name], [[1, p], [128, mb]], off))
        return t

    def load_bias_row(pool, name, M, k=None):
        """DRAM [M] (or [4,M] row k) -> [1, M] row tile."""
        t = pool.tile([1, M], FP8, tag=f"br_{name}{'' if k is None else k}")
        off = 0 if k is None else k * M
        nc.gpsimd.dma_start(out=t, in_=ap_view(io[name], [[M, 1], [1, M]], off))
        return t

    def load_w(pool, name, K, M, k=None, dt=FP8):
        kc = (K + 127) // 128
        p = min(K, 128)
        t = pool.tile([128, kc, M], dt, tag=f"w_{name}{'' if k is None else k}")
        src = io[name] if k is None else io[name][k]
        view = src.rearrange("(c p) m -> p c m", p=128) if K >= 128 else src.unsqueeze(1)
        nc.gpsimd.dma_start(out=t[:p], in_=view)
        return t

    # ================= emit all input/weight DMAs up front =================
    nbt = max(1, BL // 128)
    btr = min(BL, 128)
    fi_t = []
    for bt in range(nbt):
        fi = fip.tile([128, DL], F32, tag=f"fi{bt}")
        nc.sync.dma_start(out=fi[:btr], in_=io["f_instruction"][bt * 128:bt * 128 + btr])
        fi_t.append(fi)

    w_dec = [load_w(wdec, f"dec_W{i}", K, M) for i, (K, M) in
             enumerate([(1024, 512), (512, 256), (256, 256), (256, 128), (128, 3)], 1)]
    b_dec = [load_bias_col(wdec, f"dec_b{i}", M) for i, M in
             enumerate([512, 256, 256, 128], 1)]
    b_dec5 = load_bias_col(wpool, "dec_b5", 3)
    w_map1 = [load_w(wdec, "map_W1", 1024, 512, k) for k in range(4)]
    w_map2 = [load_w(wdec, "map_W2", 512, 256, k) for k in range(4)]
    b_map1 = [load_bias_col(wdec, "map_b1", 512, k) for k in range(4)]
    b_map2 = [load_bias_col(wdec, "map_b2", 256, k) for k in range(4)]
    w_att1 = [load_w(wpool, "att_W1", 512, 256, k) for k in range(4)]
    w_att2 = [load_w(wpool, "att_W2", 256, 256, k) for k in range(4)]
    w_att3 = [load_w(wpool, "att_W3", 256, 128, k) for k in range(4)]
    w_att4 = [load_w(wpool, "att_W4", 128, 1, k) for k in range(4)]
    b_att1r = [load_bias_row(wpool, "att_b1", 256, k) for k in range(4)]
    b_att2 = [load_bias_col(wpool, "att_b2", 256, k) for k in range(4)]
    b_att3 = [load_bias_col(wpool, "att_b3", 128, k) for k in range(4)]
    w_pred = [load_w(wpool, f"pred_W{i}", K, M) for i, (K, M) in
              enumerate([(13, 256), (256, 512), (512, 512), (512, 256), (256, 5)], 1)]
    b_pred = [load_bias_col(wpool, f"pred_b{i}", M) for i, M in
              enumerate([256, 512, 512, 256], 1)]
    b_pred5 = load_bias_col(wpool, "pred_b5", 5)

    out_pred = io["out_pred"]
    out_matched = io["out_matched"]

    # ================= gram + scan (weight-independent) =================
    foT_blks = []
    blk_tiles = {}
    blk_state = []

    def gram_pre(blk):
        scan_gram = scp.tile([128, 32, 32], F32, tag="scan_gram")
        foT = act.tile([128, 2, PB * NB], FP8, tag=f"foT{blk}")
        foT_blks.append(foT)
        blk_tiles[blk] = (scan_gram, foT)

    def gram_sub(blk, si):
        scan_gram, foT = blk_tiles[blk]
        s0 = blk * PB + si * SUB
        fo_nat = tp.tile([128, SUBG, 256], F32, tag="fo_nat")
        ff_nat = tp.tile([128, SUBG, 256], F32, tag="ff_nat")
        nc.sync.dma_start(out=fo_nat, in_=io["f_objects"][s0:s0 + SUB]
                          .rearrange("(g b) i d -> (b i) g d", b=8))
        nc.sync.dma_start(out=ff_nat, in_=io["f_objects_final"][s0:s0 + SUB]
                          .rearrange("(g b) i d -> (b i) g d", b=8))
        zt = zp.tile([128, SUB, 2, 32], F32, tag="zt")
        for g in range(SUBG):
            for c in range(2):
                # both transposes share one psum bank: ff -> cols 0:128, fo -> 128:256
                pt_full = ps_tr.tile([128, 512], F32, tag="tr")
                pe_transpose(nc, pt_full[:, 0:128],
                             ff_nat[:, g, c * 128:(c + 1) * 128], ident)
                pe_transpose(nc, pt_full[:, 128:256],
                             fo_nat[:, g, c * 128:(c + 1) * 128], ident)
                nc.vector.tensor_copy(
                    zt[:, g * 8:g * 8 + 8, c, :],
                    ap_view(pt_full, [list(pt_full.ap[0]), [16, 8], [128, 2], [1, 16]]))
                col = (si * SUB + g * 8) * NB
                nc.scalar.copy(foT[:, c, col:col + 128], pt_full[:, 128:256])
        # gram: 16 batches per psum bank; batch bb -> partitions (bb%4)*32,
        # free cols (bb//4)*32
        gram_sb = sm.tile([128, 2, 128], F32, tag="gram_sb")
        for half in range(2):
            gp_full = ps_sm.tile([128, 512], F32, tag="sm")
            gp = gp_full[:, :128]
            for bb_ in range(16):
                bb = half * 16 + bb_
                po = (bb_ % 4) * 32
                fo_ = (bb_ // 4) * 32
                for c in range(2):
                    nc.tensor.matmul(gp[po:po + 32, fo_:fo_ + 32],
                                     zt[:, bb, c, :], zt[:, bb, c, :],
                                     start=(c == 0), stop=(c == 1),
                                     tile_position=(0, po))
            if half == 0:
                nc.scalar.copy(gram_sb[:, half, :], gp)
            else:
                nc.vector.tensor_copy(gram_sb[:, half, :], gp)
        for q in range(SUB // 4):
            p0 = si * SUB + q * 4
            eng = nc.sync if q % 2 == 0 else nc.scalar
            eng.dma_start(out=scan_gram[p0:p0 + 4],
                          in_=gram_sb[:, q // 4, (q % 4) * 32:(q % 4) * 32 + 32])

    def scan_blk(blk):
        # ---------- matching scan for this block (DVE, transposed base) ----------
        scan_gram, _ = blk_tiles[blk]
        b0 = blk * PB
        gum = scp.tile([128, NB, NB], F32, tag="gum")
        nc.sync.dma_start(out=gum[:PB], in_=io["gumbel"][b0:b0 + PB])
        bfx = scp.tile([128, NB, 5], F32, tag="bfx")
        nc.sync.dma_start(out=bfx[:PB], in_=io["bboxes_f"][b0:b0 + PB])
        bix = act.tile([128, NB, 5], F32, tag=f"bix{blk}")
        nc.sync.dma_start(out=bix[:PB], in_=io["bboxes_i"][b0:b0 + PB])

        gflat = scan_gram.rearrange("p a b -> p (a b)")
        sq_ff = sm.tile([128, NB], F32, tag="sq_ff")
        nc.scalar.activation(sq_ff[:PB], ap_view(gflat, [list(gflat.ap[0]), [33, 16]])[:PB],
                             AF.Sqrt)
        inv_ff = sm.tile([128, NB], F32, tag="inv_ff")
        nc.vector.reciprocal(inv_ff[:PB], sq_ff[:PB])
        sq_fo = sm.tile([128, NB], F32, tag="sq_fo")
        nc.scalar.activation(sq_fo[:PB], ap_view(gflat, [list(gflat.ap[0]), [33, 16]], 528)[:PB],
                             AF.Sqrt)
        inv_fo = sm.tile([128, NB], F32, tag="inv_fo")
        nc.vector.reciprocal(inv_fo[:PB], sq_fo[:PB])

        # baseT[p, i, j] = rho[i,j] + gumbel[i,j]  (rho from gram block [16:32, 0:16])
        baseT = scp.tile([128, NB, NB], F32, tag="baseT")
        nc.vector.tensor_tensor(out=baseT[:PB], in0=scan_gram[:PB, 16:32, 0:16],
                                in1=inv_fo[:PB].unsqueeze(2).broadcast_to([PB, 16, 16]),
                                op=ALU.mult)
        nc.vector.tensor_tensor(out=baseT[:PB], in0=baseT[:PB],
                                in1=inv_ff[:PB].unsqueeze(1).broadcast_to([PB, 16, 16]),
                                op=ALU.mult)
        nc.vector.tensor_tensor(out=baseT[:PB], in0=baseT[:PB], in1=gum[:PB], op=ALU.add)

        pen = sm.tile([128, NB], F32, tag="pen")
        nc.vector.memset(pen[:PB], 0.0)
        pmat = scp.tile([128, NB, NB], F32, tag="pmat")
        for i in range(NB):
            score = sm.tile([128, NB], F32, tag="score")
            nc.vector.tensor_tensor(out=score[:PB], in0=baseT[:PB, i, :], in1=pen[:PB],
                                    op=ALU.add)
            mx8 = sm.tile([128, 8], F32, tag="mx8")
            nc.vector.max(mx8[:PB], score[:PB])
            oh = sm.tile([128, NB], F32, tag="oh")
            nc.vector.tensor_tensor(out=oh[:PB], in0=score[:PB],
                                    in1=mx8[:PB, 0:1].broadcast_to([PB, NB]),
                                    op=ALU.is_equal)
            nc.vector.scalar_tensor_tensor(out=pen[:PB], in0=oh[:PB], scalar=-1e5,
                                           in1=pen[:PB], op0=ALU.mult, op1=ALU.add)
            nc.vector.tensor_copy(pmat[:PB, i, :], oh[:PB])
        boxes = scp.tile([128, NB, 5], F32, tag="boxes")
        for c_ in range(5):
            prod = sm.tile([128, NB, NB], F32, tag="bprod")
            nc.vector.tensor_tensor(out=prod[:PB], in0=pmat[:PB],
                                    in1=bfx[:PB, :, c_].unsqueeze(1)
                                    .broadcast_to([PB, NB, NB]), op=ALU.mult)
            nc.vector.reduce_sum(boxes[:PB, :, c_], prod[:PB], axis=AX.X)
        nc.sync.dma_start(out=out_matched[b0:b0 + PB], in_=boxes[:PB])
        blk_state.append(bix)

    for blk in range(NBLK):
        gram_pre(blk)
        for si in range(NSUB_B):
            gram_sub(blk, si)
        scan_blk(blk)

    # ================= dec/map MLPs (dense PE, warms clock) =================
    finstT = act.tile([128, 8, BL], FP8)
    for bt in range(nbt):
        for kc in range(8):
            pt_full = ps_tr.tile([128, 512], F32, tag="tr")
            pt = pt_full[:, :128]
            pe_transpose(nc, pt[:, :btr], fi_t[bt][:btr, kc * 128:(kc + 1) * 128],
                         ident[:btr, :btr])
            if kc % 2 == 0:
                nc.scalar.copy(finstT[:, kc, bt * 128:bt * 128 + btr], pt[:, :btr])
            else:
                nc.vector.tensor_copy(finstT[:, kc, bt * 128:bt * 128 + btr], pt[:, :btr])

    def mm_dr(ps, w, x, kc, m, mw, n_lo, n_hi):
        """psum[:mw, :] += sum_c w[:,c,m128:+mw].T @ x[:,c,n_lo:n_hi], DoubleRow pairs."""
        kc2 = kc // 2
        for c2 in range(kc2):
            nc.tensor.matmul(ps[:mw], w[:, 2 * c2:2 * c2 + 2, m * 128:m * 128 + mw],
                             x[:, 2 * c2:2 * c2 + 2, n_lo:n_hi], perf_mode=DR,
                             start=(c2 == 0), stop=(c2 == kc2 - 1))

    def mlp_layer(pool, x, w, b, kc, M, act_fn=AF.Relu, tag="h"):
        """x: [128, kc, BL] fp8 -> out [128, M/128, BL] fp8 via DR matmuls + ACT."""
        mb = (M + 127) // 128
        o = pool.tile([128, mb, BL], FP8, tag=tag)
        for m in range(mb):
            mw = min(128, M - m * 128)
            ps_full = ps_mm.tile([128, 1024], F32, tag="mm")
            ps = ps_full[:, :BL]
            mm_dr(ps, w, x, kc, m, mw, 0, BL)
            if m % 2 == 0:
                nc.scalar.activation(o[:mw, m, :], ps[:mw], act_fn, bias=b[:mw, m:m + 1])
            elif act_fn == AF.Relu:
                nc.vector.tensor_scalar(out=o[:mw, m, :], in0=ps[:mw],
                                        scalar1=b[:mw, m:m + 1], scalar2=0.0,
                                        op0=ALU.add, op1=ALU.max)
            else:
                nc.vector.tensor_scalar(out=o[:mw, m, :], in0=ps[:mw],
                                        scalar1=b[:mw, m:m + 1], scalar2=None,
                                        op0=ALU.add)
        return o

    # ---------- dec MLP -> f_action -> emb[0:3] ----------
    emb = act.tile([16, BL], FP8)
    h = finstT
    for li, (K, M) in enumerate([(1024, 512), (512, 256), (256, 256), (256, 128)]):
        h = mlp_layer(hp_big, h, w_dec[li], b_dec[li], K // 128, M, tag="dec")
    ps5_full = ps_mm.tile([128, 1024], F32, tag="mm")
    ps5 = ps5_full[:, :BL]
    nc.tensor.matmul(ps5[:3], w_dec[4][:, 0, 0:3], h[:, 0, :], start=True, stop=True)
    e_sb = sm.tile([3, BL], F32, tag="e_sb")
    nc.scalar.activation(e_sb, ps5[:3], AF.Exp, bias=b_dec5[:3, 0:1])
    ps_s_full = ps_sm.tile([128, 512], F32, tag="sm")
    ps_s = ps_s_full[:, :BL]
    nc.tensor.matmul(ps_s[:1], ones[:3, 0:1], e_sb[:], start=True, stop=True)
    r_sb = sm.tile([1, BL], F32, tag="r_sb")
    nc.vector.reciprocal(r_sb, ps_s[:1])
    ps_rb_full = ps_sm.tile([128, 512], F32, tag="sm")
    ps_rb = ps_rb_full[:, :BL]
    nc.tensor.matmul(ps_rb[:3], ones[0:1, 0:3], r_sb[:], start=True, stop=True)
    nc.vector.tensor_tensor(out=emb[0:3], in0=e_sb[:], in1=ps_rb[:3], op=ALU.mult)

    # ---------- map MLPs -> c_kT [batch, feat] bf16 (includes b_att1) ----------
    ckT = []
    for k in range(4):
        h1m = mlp_layer(hp_big, finstT, w_map1[k], b_map1[k], 8, 512, tag="map1")
        mk = mlp_layer(hp_big, h1m, w_map2[k], b_map2[k], 4, 256,
                       act_fn=AF.Identity, tag="mk")
        # ckt_t: [128 batch-in-block, nblk, 256 feat], includes b_att1
        ckt_t = act.tile([128, BL // 128, 256], BF16, tag=f"ckT{k}")
        for bblk in range(BL // 128):
            ps_full = ps_sm.tile([128, 512], F32, tag="sm")
            ps = ps_full[:, :256]
            nc.tensor.matmul(ps, mk[:, 0:2, bblk * 128:bblk * 128 + 128],
                             w_att1[k][:, 0:2, 0:256], perf_mode=DR,
                             start=True, stop=False)
            nc.tensor.matmul(ps, ones1[0:1, :128], b_att1r[k][0:1, :],
                             start=False, stop=True)
            if bblk % 2 == 0:
                nc.scalar.copy(ckt_t[:, bblk, :], ps)
            else:
                nc.vector.tensor_copy(ckt_t[:, bblk, :], ps)
        ckT.append(ckt_t)

    # ================= attention =================
    def att_rt(blk, rt, s4all):
        foT = foT_blks[blk]
        col0 = rt * RT
        ps4_full = ps_sm.tile([128, 512], F32, tag="sm")
        ps4 = ps4_full[:, :RT]
        for k in range(4):
            # --- att1: obj part (fp8 DR) + c_k via mask matmul ---
            ps1_full = ps_mm.tile([128, 1024], F32, tag="mm")
            h1 = h1p.tile([128, 2, RT], FP8, tag="h1")
            for m in range(2):
                ps = ps1_full[:, m * RT:(m + 1) * RT]
                nc.tensor.matmul(ps, w_att1[k][:, 2:4, m * 128:(m + 1) * 128],
                                 foT[:, 0:2, col0:col0 + RT], perf_mode=DR,
                                 start=True, stop=False)
                nc.tensor.matmul(ps, ckT[k][rt * 32:(rt + 1) * 32, blk, m * 128:(m + 1) * 128],
                                 mask128[rt * 32:(rt + 1) * 32].rearrange("p a b -> p (a b)"),
                                 start=False, stop=True,
                                 tile_position=(rt * 32, 0))
                if m == 0:
                    nc.scalar.activation(h1[:, m, :], ps, AF.Relu)
                else:
                    nc.vector.tensor_scalar(out=h1[:, m, :], in0=ps,
                                            scalar1=0.0, scalar2=None, op0=ALU.max)
            # --- att2 (fp8 DR) ---
            ps2_full = ps_mm.tile([128, 1024], F32, tag="mm")
            h2 = h2p.tile([128, 2, RT], FP8, tag="h2")
            for m in range(2):
                ps = ps2_full[:, m * RT:(m + 1) * RT]
                nc.tensor.matmul(ps, w_att2[k][:, 0:2, m * 128:(m + 1) * 128],
                                 h1[:, 0:2, :], perf_mode=DR, start=True, stop=True)
                if m == 0:
                    nc.scalar.activation(h2[:, m, :], ps, AF.Relu,
                                         bias=b_att2[k][:, m:m + 1])
                else:
                    nc.vector.tensor_scalar(out=h2[:, m, :], in0=ps,
                                            scalar1=b_att2[k][:, m:m + 1],
                                            scalar2=0.0, op0=ALU.add, op1=ALU.max)
            # --- att3 (fp8 DR) ---
            ps3_full = ps_tr.tile([128, 512], F32, tag="tr")
            ps3 = ps3_full[:, :RT]
            nc.tensor.matmul(ps3, w_att3[k][:, 0:2, 0:128], h2[:, 0:2, :],
                             perf_mode=DR, start=True, stop=True)
            h3 = h3p.tile([128, RT], FP8, tag="h3")
            nc.scalar.activation(h3, ps3, AF.Relu, bias=b_att3[k][:, 0:1])
            # --- att4 -> scores at psum partition 32k ---
            nc.tensor.matmul(ps4[32 * k:32 * k + 1, :], w_att4[k][:, 0, 0:1],
                             h3, start=True, stop=True,
                             tile_position=(0, 32 * k))
        nc.vector.tensor_copy(s4all[:, rt, :], ps4)

    def att_post(blk):
        b0 = blk * PB
        bix = blk_state[blk]
        s4all = att_s4[blk]
        sTk = []
        for k in range(4):
            t = sm.tile([128, NB], F32, tag=f"sTk{k}")
            eng = nc.sync if k % 2 == 0 else nc.scalar
            eng.dma_start(out=t[:PB], in_=s4all[32 * k:32 * k + 1, :, :])
            sTk.append(t)

        def softmax_pair(ka, kb, tag):
            z = sm.tile([128, NB], F32, tag=f"z{tag}")
            nc.vector.tensor_tensor(out=z[:PB], in0=sTk[ka][:PB], in1=sTk[kb][:PB],
                                    op=ALU.add)
            e = sm.tile([128, NB], F32, tag=f"e{tag}")
            nc.scalar.activation(e[:PB], z[:PB], AF.Exp)
            ssum = sm.tile([128, 1], F32, tag=f"ss{tag}")
            nc.vector.reduce_sum(ssum[:PB], e[:PB], axis=AX.X)
            rinv = sm.tile([128, 1], F32, tag=f"ri{tag}")
            nc.vector.reciprocal(rinv[:PB], ssum[:PB])
            a = act.tile([128, NB], F32, tag=f"a{tag}{blk}")
            nc.vector.tensor_scalar(out=a[:PB], in0=e[:PB], scalar1=rinv[:PB, 0:1],
                                    scalar2=None, op0=ALU.mult)
            return a

        a_sub = softmax_pair(0, 1, "s")
        a_obj = softmax_pair(2, 3, "o")

        def weighted_loc(a, tag):
            prod = sm.tile([128, 5, NB], F32, tag=f"lp{tag}")
            nc.vector.tensor_tensor(out=prod[:PB], in0=bix[:PB].transpose([0, 2, 1]),
                                    in1=a[:PB].unsqueeze(1).broadcast_to([PB, 5, NB]),
                                    op=ALU.mult)
            loc = sm.tile([128, 5], F32, tag=f"loc{tag}")
            nc.vector.reduce_sum(loc[:PB], prod[:PB], axis=AX.X)
            return loc

        sloc = weighted_loc(a_sub, "s")
        oloc = weighted_loc(a_obj, "o")
        for loc, r0 in ((oloc, 3), (sloc, 8)):
            pt_full = ps_tr.tile([128, 512], F32, tag="tr")
            pt = pt_full[:, :128]
            pe_transpose(nc, pt[:5, :PB], loc[:PB], ident[:PB, :PB])
            locT = sm.tile([5, 128], FP8, tag="locT")
            nc.scalar.copy(locT[:, :PB], pt[:5, :PB])
            nc.sync.dma_start(out=emb[r0:r0 + 5, b0:b0 + PB], in_=locT[:, :PB])
        blk_state[blk] = (a_sub, bix)

    att_s4 = {}
    for blk in range(NBLK):
        s4all = sm.tile([128, NRT_B, RT], F32, tag="s4all")
        att_s4[blk] = s4all
        for rt in range(NRT_B):
            att_rt(blk, rt, s4all)
        att_post(blk)

    # ================= pred MLP + blend =================
    p1 = hp_big.tile([128, 2, BL], FP8, tag="pred")
    for m in range(2):
        ps_full = ps_mm.tile([128, 1024], F32, tag="mm")
        ps = ps_full[:, :BL]
        nc.tensor.matmul(ps, w_pred[0][:13, 0, m * 128:(m + 1) * 128], emb[:13],
                         start=True, stop=True)
        if m == 0:
            nc.scalar.activation(p1[:, m, :], ps, AF.Relu, bias=b_pred[0][:, m:m + 1])
        else:
            nc.vector.tensor_scalar(out=p1[:, m, :], in0=ps,
                                    scalar1=b_pred[0][:, m:m + 1], scalar2=0.0,
                                    op0=ALU.add, op1=ALU.max)
    h = p1
    for li, (K, M) in enumerate([(256, 512), (512, 512), (512, 256)], 1):
        h = mlp_layer(hp_big, h, w_pred[li], b_pred[li], K // 128, M, tag="pred")
    ps5_full = ps_sm.tile([128, 512], F32, tag="sm")
    ps5 = ps5_full[:, :BL]
    for c in range(2):
        nc.tensor.matmul(ps5[:5], w_pred[4][:, c, 0:5], h[:, c, :],
                         start=(c == 0), stop=(c == 1))
    predv = act.tile([5, BL], F32)
    nc.scalar.activation(predv, ps5[:5], AF.Tanh, bias=b_pred5[:5, 0:1])

    for blk in range(NBLK):
        b0 = blk * PB
        a_sub, bix = blk_state[blk]
        pt_full = ps_tr.tile([128, 512], F32, tag="tr")
        pt = pt_full[:, :128]
        pe_transpose(nc, pt[:PB, :5], predv[:, b0:b0 + PB], ident[:5, :5])
        predT = sm.tile([128, 5], F32, tag="predT")
        nc.vector.tensor_copy(predT[:PB], pt[:PB, :5])
        d = scp.tile([128, NB, 5], F32, tag="d")
        nc.vector.tensor_tensor(out=d[:PB], in0=predT[:PB].unsqueeze(1)
                                .broadcast_to([PB, NB, 5]), in1=bix[:PB], op=ALU.subtract)
        nc.vector.tensor_tensor(out=d[:PB], in0=d[:PB],
                                in1=a_sub[:PB].unsqueeze(2).broadcast_to([PB, NB, 5]),
                                op=ALU.mult)
        outb = scp.tile([128, NB, 5], F32, tag="outb")
        nc.vector.tensor_tensor(out=outb[:PB], in0=d[:PB], in1=bix[:PB], op=ALU.add)
        nc.sync.dma_start(out=out_pred[b0:b0 + PB], in_=outb[:PB])

    ctx.close()


INPUT_SPECS = [
    ("f_objects", (NB, DV)), ("f_objects_final", (NB, DV)),
    ("bboxes_i", (NB, 5)), ("bboxes_f", (NB, 5)),
    ("f_instruction", (DL,)), ("gumbel", (NB, NB)),
]
WEIGHT_SPECS = (
    [(f"dec_W{i}", s) for i, s in enumerate([(1024, 512), (512, 256), (256, 256), (256, 128), (128, 3)], 1)]
    + [(f"dec_b{i}", (s,)) for i, s in enumerate([512, 256, 256, 128, 3], 1)]
    + [("map_W1", (4, 1024, 512)), ("map_b1", (4, 512)), ("map_W2", (4, 512, 256)), ("map_b2", (4, 256))]
    + [(f"att_W{i}", (4,) + s) for i, s in enumerate([(512, 256), (256, 256), (256, 128), (128, 1)], 1)]
    + [(f"att_b{i}", (4, s)) for i, s in enumerate([256, 256, 128, 1], 1)]
    + [(f"pred_W{i}", s) for i, s in enumerate([(13, 256), (256, 512), (512, 512), (512, 256), (256, 5)], 1)]
    + [(f"pred_b{i}", (s,)) for i, s in enumerate([256, 512, 512, 256, 5], 1)]
)


def declare_io(nc, BL):
    io = {}
    for name, tail in INPUT_SPECS:
        io[name] = nc.dram_tensor(name, [BL] + list(tail), F32, kind="ExternalInput").ap()
    for name, shape in WEIGHT_SPECS:
        io[name] = nc.dram_tensor(name, list(shape), F32, kind="ExternalInput").ap()
    io["out_pred"] = nc.dram_tensor("out_pred", [BL, NB, 5], F32, kind="ExternalOutput").ap()
    io["out_matched"] = nc.dram_tensor("out_matched", [BL, NB, 5], F32, kind="ExternalOutput").ap()
    return io
# ======================= SPMD driver =======================
import numpy as np

N_CORES = 8
B_FULL = 4096
BL_CORE = B_FULL // N_CORES

_BATCH_INPUTS = ("f_objects", "f_objects_final", "bboxes_i", "bboxes_f",
                 "f_instruction", "gumbel")

_NC = None


def _get_nc():
    global _NC
    if _NC is None:
        from concourse import bacc
        import concourse.tile as tile
        nc = bacc.Bacc("TRN2", target_bir_lowering=False, debug=False,
                       num_devices=N_CORES)
        io = declare_io(nc, BL_CORE)
        with tile.TileContext(nc) as tc:
            build_kernel(tc, io, BL_CORE)
        nc.compile()
        _NC = nc
    return _NC


def kernel(**inputs):
    from concourse.bass_utils import run_bass_kernel_spmd
    nc = _get_nc()
    arrs = {k: np.ascontiguousarray(np.asarray(v, dtype=np.float32))
            for k, v in inputs.items()}
    in_maps = []
    for c in range(N_CORES):
        m = {}
        for k, v in arrs.items():
            if k in _BATCH_INPUTS:
                m[k] = v[c * BL_CORE:(c + 1) * BL_CORE]
            else:
                m[k] = v
        in_maps.append(m)
    res = run_bass_kernel_spmd(nc, in_maps, list(range(N_CORES)))
    pred = np.concatenate([res.results[c]["out_pred"] for c in range(N_CORES)], axis=0)
    matched = np.concatenate([res.results[c]["out_matched"] for c in range(N_CORES)], axis=0)
    return pred, matched


# revision 29
# speedup vs baseline: 971.4755x; 1.1111x over previous
"""Bass/Tile kernel for nn_BaselineModel (gumbel matching + attention MLPs).

v2 layout/precision plan:
  - MLPs in [feature-on-partition, row-on-free], fp8e4 operands with DoubleRow
    matmuls (K>=256), f32 psum. Matching stays f32 (argmax-flip safety).
  - Per-batch map-branch contribution c_k enters the att1 PSUM group via a
    K=32 one-hot "mask matmul" (batch -> 16 rows), so h1 is a single ACT
    Relu from PSUM (no gpsimd, no DVE broadcast add).
  - att4 scores for the 4 branches land in one PSUM bank at partitions
    {0,32,64,96} (tile_position col groups); one copy + one DMA per (k,blk)
    delivers contiguous [batch, object] score tiles. b_att4 is dropped
    (softmax shift invariance).
  - Matching scan runs on the transposed rho block (free transpose from the
    symmetric Z Z^T gram), so gumbel needs no transpose and every DVE op in
    the scan is contiguous. Matched boxes = accumulated one-hot matrix
    applied post-loop.
"""
import sys
sys.path.insert(0, "/opt/trn_rl_repo")
from contextlib import ExitStack
import concourse.bass as bass
import concourse.mybir as mybir
from concourse.masks import make_identity

F32 = mybir.dt.float32
BF16 = mybir.dt.bfloat16
FP8 = mybir.dt.float8e4
AF = mybir.ActivationFunctionType
ALU = mybir.AluOpType
AX = mybir.AxisListType
DR = mybir.MatmulPerfMode.DoubleRow

NB = 16    # objects per batch
DV = 256   # visual feature dim
DL = 1024  # instruction dim


def ap_view(ap, dims, extra_offset=0):
    return bass.AP(tensor=ap.tensor, offset=ap.offset + extra_offset, ap=list(dims))


def pe_transpose(nc, out, in_, ident):
    return nc.tensor.matmul(out, in_, ident, is_transpose=True, start=True, stop=True)


def build_kernel(tc, io, BL):
    """io: dict name -> DRAM AP (inputs + out_pred, out_matched). BL: batches/core.

    Phase order (v4): input/weight DMAs queue first; gram+scan (weight-
    independent) runs under the DMA stream; then dec/map MLPs (dense PE,
    warms the clock); then attention; then pred+blend.
    """
    nc = tc.nc
    assert BL % 32 == 0
    SUB = 32                    # gram sub-block (batches)
    PB = min(128, BL)           # scan block (batches)
    NBLK = BL // PB
    NSUB_B = PB // SUB          # subs per block
    SUBG = SUB // 8             # groups of 8 batches per sub
    RT = 512                    # rows per attention tile
    BRT = RT // NB              # batches per attention row tile (32)
    NRT_B = PB // BRT           # row tiles per block

    ctx = ExitStack()

    # ---------- pools ----------
    wpool = ctx.enter_context(tc.tile_pool(name="wpool", bufs=1))
    act = ctx.enter_context(tc.tile_pool(name="act", bufs=1))
    hp_big = ctx.enter_context(tc.tile_pool(name="hp_big", bufs=2))
    sm = ctx.enter_context(tc.tile_pool(name="sm", bufs=2))
    scp = ctx.enter_context(tc.tile_pool(name="scp", bufs=2))
    tp = ctx.enter_context(tc.tile_pool(name="tp", bufs=2))
    zp = ctx.enter_context(tc.tile_pool(name="zp", bufs=2))
    h1p = ctx.enter_context(tc.tile_pool(name="h1p", bufs=4))
    h2p = ctx.enter_context(tc.tile_pool(name="h2p", bufs=4))
    h3p = ctx.enter_context(tc.tile_pool(name="h3p", bufs=3))
    fip = ctx.enter_context(tc.tile_pool(name="fip", bufs=1))
    wdec = ctx.enter_context(tc.tile_pool(name="wdec", bufs=1))
    ps_mm = ctx.enter_context(tc.tile_pool(name="ps_mm", bufs=2, space="PSUM"))
    ps_sm = ctx.enter_context(tc.tile_pool(name="ps_sm", bufs=2, space="PSUM"))
    ps_tr = ctx.enter_context(tc.tile_pool(name="ps_tr", bufs=2, space="PSUM"))

    ident = wpool.tile([128, 128], F32)
    make_identity(nc, ident)
    ones = wpool.tile([16, 16], F32)
    nc.vector.memset(ones, 1.0)
    ones1 = wpool.tile([1, 128], FP8)
    nc.vector.memset(ones1, 1.0)
    # one-hot batch->row expansion: mask128[32g + j, b*NB + n] = (b == j)
    mask128 = wpool.tile([128, 32, NB], BF16)
    for g in range(4):
        nc.vector.tensor_copy(
            mask128[g * 32:(g + 1) * 32],
            ident[g * 32:(g + 1) * 32, g * 32:(g + 1) * 32].unsqueeze(2)
            .broadcast_to([32, 32, NB]))

    def load_bias_col(pool, name, M, k=None):
        """DRAM [M] (or [4,M] row k) -> [p, mb] tile, column m = feats m*128..."""
        mb = (M + 127) // 128
        p = min(M, 128)
        t = pool.tile([128, mb], F32, tag=f"b_{name}{'' if k is None else k}")
        off = 0 if k is None else k * M
        nc.scalar.dma_start(out=t[:p, :], in_=ap_view(io[name], [[1, p], [128, mb]], off))
        return t

    def load_bias_row(pool, name, M, k=None):
        """DRAM [M] (or [4,M] row k) -> [1, M] row tile."""
        t = pool.tile([1, M], FP8, tag=f"br_{name}{'' if k is None else k}")
        off = 0 if k is None else k * M
        nc.gpsimd.dma_start(out=t, in_=ap_view(io[name], [[M, 1], [1, M]], off))
        return t

    def load_w(pool, name, K, M, k=None, dt=FP8):
        kc = (K + 127) // 128
        p = min(K, 128)
        t = pool.tile([128, kc, M], dt, tag=f"w_{name}{'' if k is None else k}")
        src = io[name] if k is None else io[name][k]
        view = src.rearrange("(c p) m -> p c m", p=128) if K >= 128 else src.unsqueeze(1)
        nc.gpsimd.dma_start(out=t[:p], in_=view)
        return t

    # ================= emit all input/weight DMAs up front =================
    nbt = max(1, BL // 128)
    btr = min(BL, 128)
    fi_t = []
    for bt in range(nbt):
        fi = fip.tile([128, DL], F32, tag=f"fi{bt}")
        nc.sync.dma_start(out=fi[:btr], in_=io["f_instruction"][bt * 128:bt * 128 + btr])
        fi_t.append(fi)

    w_dec = [load_w(wdec, f"dec_W{i}", K, M) for i, (K, M) in
             enumerate([(1024, 512), (512, 256), (256, 256), (256, 128), (128, 3)], 1)]
    b_dec = [load_bias_col(wdec, f"dec_b{i}", M) for i, M in
             enumerate([512, 256, 256, 128], 1)]
    b_dec5 = load_bias_col(wpool, "dec_b5", 3)
    w_map1 = [load_w(wdec, "map_W1", 1024, 512, k) for k in range(4)]
    w_map2 = [load_w(wdec, "map_W2", 512, 256, k) for k in range(4)]
    b_map1 = [load_bias_col(wdec, "map_b1", 512, k) for k in range(4)]
    b_map2 = [load_bias_col(wdec, "map_b2", 256, k) for k in range(4)]
    w_att1 = [load_w(wpool, "att_W1", 512, 256, k) for k in range(4)]
    w_att2 = [load_w(wpool, "att_W2", 256, 256, k) for k in range(4)]
    w_att3 = [load_w(wpool, "att_W3", 256, 128, k) for k in range(4)]
    w_att4 = [load_w(wpool, "att_W4", 128, 1, k) for k in range(4)]
    b_att1r = [load_bias_row(wpool, "att_b1", 256, k) for k in range(4)]
    b_att2 = [load_bias_col(wpool, "att_b2", 256, k) for k in range(4)]
    b_att3 = [load_bias_col(wpool, "att_b3", 128, k) for k in range(4)]
    w_pred = [load_w(wpool, f"pred_W{i}", K, M) for i, (K, M) in
              enumerate([(13, 256), (256, 512), (512, 512), (512, 256), (256, 5)], 1)]
    b_pred = [load_bias_col(wpool, f"pred_b{i}", M) for i, M in
              enumerate([256, 512, 512, 256], 1)]
    b_pred5 = load_bias_col(wpool, "pred_b5", 5)

    out_pred = io["out_pred"]
    out_matched = io["out_matched"]

    # ================= gram + scan (weight-independent) =================
    foT_blks = []
    blk_tiles = {}
    blk_state = []

    def gram_pre(blk):
        scan_gram = scp.tile([128, 32, 32], F32, tag="scan_gram")
        foT = act.tile([128, 2, PB * NB], FP8, tag=f"foT{blk}")
        foT_blks.append(foT)
        blk_tiles[blk] = (scan_gram, foT)

    def gram_transposes(blk, si):
        scan_gram, foT = blk_tiles[blk]
        s0 = blk * PB + si * SUB
        fo_nat = tp.tile([128, SUBG, 256], F32, tag="fo_nat")
        ff_nat = tp.tile([128, SUBG, 256], F32, tag="ff_nat")
        nc.sync.dma_start(out=fo_nat, in_=io["f_objects"][s0:s0 + SUB]
                          .rearrange("(g b) i d -> (b i) g d", b=8))
        nc.sync.dma_start(out=ff_nat, in_=io["f_objects_final"][s0:s0 + SUB]
                          .rearrange("(g b) i d -> (b i) g d", b=8))
        zt = zp.tile([128, SUB, 2, 32], F32, tag="zt")
        for g in range(SUBG):
            for c in range(2):
                # both transposes share one psum bank: ff -> cols 0:128, fo -> 128:256
                pt_full = ps_tr.tile([128, 512], F32, tag="tr")
                pe_transpose(nc, pt_full[:, 0:128],
                             ff_nat[:, g, c * 128:(c + 1) * 128], ident)
                pe_transpose(nc, pt_full[:, 128:256],
                             fo_nat[:, g, c * 128:(c + 1) * 128], ident)
                nc.vector.tensor_copy(
                    zt[:, g * 8:g * 8 + 8, c, :],
                    ap_view(pt_full, [list(pt_full.ap[0]), [16, 8], [128, 2], [1, 16]]))
                col = (si * SUB + g * 8) * NB
                nc.scalar.copy(foT[:, c, col:col + 128], pt_full[:, 128:256])
        return zt

    def gram_mms(blk, si, zt):
        # gram: 16 batches per psum bank; batch bb -> partitions (bb%4)*32,
        # free cols (bb//4)*32
        scan_gram, _ = blk_tiles[blk]
        gram_sb = sm.tile([128, 2, 128], F32, tag="gram_sb")
        for half in range(2):
            gp_full = ps_sm.tile([128, 512], F32, tag="sm")
            gp = gp_full[:, :128]
            for bb_ in range(16):
                bb = half * 16 + bb_
                po = (bb_ % 4) * 32
                fo_ = (bb_ // 4) * 32
                for c in range(2):
                    nc.tensor.matmul(gp[po:po + 32, fo_:fo_ + 32],
                                     zt[:, bb, c, :], zt[:, bb, c, :],
                                     start=(c == 0), stop=(c == 1),
                                     tile_position=(0, po))
            if half == 0:
                nc.scalar.copy(gram_sb[:, half, :], gp)
            else:
                nc.vector.tensor_copy(gram_sb[:, half, :], gp)
        for q in range(SUB // 4):
            p0 = si * SUB + q * 4
            eng = nc.sync if q % 2 == 0 else nc.scalar
            eng.dma_start(out=scan_gram[p0:p0 + 4],
                          in_=gram_sb[:, q // 4, (q % 4) * 32:(q % 4) * 32 + 32])

    def scan_blk(blk):
        # ---------- matching scan for this block (DVE, transposed base) ----------
        scan_gram, _ = blk_tiles[blk]
        b0 = blk * PB
        gum = scp.tile([128, NB, NB], F32, tag="gum")
        nc.sync.dma_start(out=gum[:PB], in_=io["gumbel"][b0:b0 + PB])
        bfx = scp.tile([128, NB, 5], F32, tag="bfx")
        nc.sync.dma_start(out=bfx[:PB], in_=io["bboxes_f"][b0:b0 + PB])
        bix = act.tile([128, NB, 5], F32, tag=f"bix{blk}")
        nc.sync.dma_start(out=bix[:PB], in_=io["bboxes_i"][b0:b0 + PB])

        gflat = scan_gram.rearrange("p a b -> p (a b)")
        sq_ff = sm.tile([128, NB], F32, tag="sq_ff")
        nc.scalar.activation(sq_ff[:PB], ap_view(gflat, [list(gflat.ap[0]), [33, 16]])[:PB],
                             AF.Sqrt)
        inv_ff = sm.tile([128, NB], F32, tag="inv_ff")
        nc.vector.reciprocal(inv_ff[:PB], sq_ff[:PB])
        sq_fo = sm.tile([128, NB], F32, tag="sq_fo")
        nc.scalar.activation(sq_fo[:PB], ap_view(gflat, [list(gflat.ap[0]), [33, 16]], 528)[:PB],
                             AF.Sqrt)
        inv_fo = sm.tile([128, NB], F32, tag="inv_fo")
        nc.vector.reciprocal(inv_fo[:PB], sq_fo[:PB])

        # baseT[p, i, j] = rho[i,j] + gumbel[i,j]  (rho from gram block [16:32, 0:16])
        baseT = scp.tile([128, NB, NB], F32, tag="baseT")
        nc.vector.tensor_tensor(out=baseT[:PB], in0=scan_gram[:PB, 16:32, 0:16],
                                in1=inv_fo[:PB].unsqueeze(2).broadcast_to([PB, 16, 16]),
                                op=ALU.mult)
        nc.vector.tensor_tensor(out=baseT[:PB], in0=baseT[:PB],
                                in1=inv_ff[:PB].unsqueeze(1).broadcast_to([PB, 16, 16]),
                                op=ALU.mult)
        nc.vector.tensor_tensor(out=baseT[:PB], in0=baseT[:PB], in1=gum[:PB], op=ALU.add)

        pen = sm.tile([128, NB], F32, tag="pen")
        nc.vector.memset(pen[:PB], 0.0)
        pmat = scp.tile([128, NB, NB], F32, tag="pmat")
        for i in range(NB):
            score = sm.tile([128, NB], F32, tag="score")
            nc.vector.tensor_tensor(out=score[:PB], in0=baseT[:PB, i, :], in1=pen[:PB],
                                    op=ALU.add)
            mx8 = sm.tile([128, 8], F32, tag="mx8")
            nc.vector.max(mx8[:PB], score[:PB])
            oh = sm.tile([128, NB], F32, tag="oh")
            nc.vector.tensor_tensor(out=oh[:PB], in0=score[:PB],
                                    in1=mx8[:PB, 0:1].broadcast_to([PB, NB]),
                                    op=ALU.is_equal)
            nc.vector.scalar_tensor_tensor(out=pen[:PB], in0=oh[:PB], scalar=-1e5,
                                           in1=pen[:PB], op0=ALU.mult, op1=ALU.add)
            nc.vector.tensor_copy(pmat[:PB, i, :], oh[:PB])
        boxes = scp.tile([128, NB, 5], F32, tag="boxes")
        for c_ in range(5):
            prod = sm.tile([128, NB, NB], F32, tag="bprod")
            nc.vector.tensor_tensor(out=prod[:PB], in0=pmat[:PB],
                                    in1=bfx[:PB, :, c_].unsqueeze(1)
                                    .broadcast_to([PB, NB, NB]), op=ALU.mult)
            nc.vector.reduce_sum(boxes[:PB, :, c_], prod[:PB], axis=AX.X)
        nc.sync.dma_start(out=out_matched[b0:b0 + PB], in_=boxes[:PB])
        blk_state.append(bix)

    # software-pipeline: transposes of (blk, si) overlap gram MMs of the
    # previous sub, so the PE queue never head-blocks on the zt copy.
    pend = None
    for blk in range(NBLK):
        gram_pre(blk)
        for si in range(NSUB_B):
            zt = gram_transposes(blk, si)
            if pend is not None:
                gram_mms(*pend)
                if pend[1] == NSUB_B - 1:
                    scan_blk(pend[0])
            pend = (blk, si, zt)
    gram_mms(*pend)
    scan_blk(NBLK - 1)

    # ================= dec/map MLPs (dense PE, warms clock) =================
    finstT = act.tile([128, 8, BL], FP8)
    for bt in range(nbt):
        for kc in range(8):
            pt_full = ps_tr.tile([128, 512], F32, tag="tr")
            pt = pt_full[:, :128]
            pe_transpose(nc, pt[:, :btr], fi_t[bt][:btr, kc * 128:(kc + 1) * 128],
                         ident[:btr, :btr])
            if kc % 2 == 0:
                nc.scalar.copy(finstT[:, kc, bt * 128:bt * 128 + btr], pt[:, :btr])
            else:
                nc.vector.tensor_copy(finstT[:, kc, bt * 128:bt * 128 + btr], pt[:, :btr])

    def mm_dr(ps, w, x, kc, m, mw, n_lo, n_hi):
        """psum[:mw, :] += sum_c w[:,c,m128:+mw].T @ x[:,c,n_lo:n_hi], DoubleRow pairs."""
        kc2 = kc // 2
        for c2 in range(kc2):
            nc.tensor.matmul(ps[:mw], w[:, 2 * c2:2 * c2 + 2, m * 128:m * 128 + mw],
                             x[:, 2 * c2:2 * c2 + 2, n_lo:n_hi], perf_mode=DR,
                             start=(c2 == 0), stop=(c2 == kc2 - 1))

    def mlp_layer(pool, x, w, b, kc, M, act_fn=AF.Relu, tag="h"):
        """x: [128, kc, BL] fp8 -> out [128, M/128, BL] fp8 via DR matmuls + ACT."""
        mb = (M + 127) // 128
        o = pool.tile([128, mb, BL], FP8, tag=tag)
        for m in range(mb):
            mw = min(128, M - m * 128)
            ps_full = ps_mm.tile([128, 1024], F32, tag="mm")
            ps = ps_full[:, :BL]
            mm_dr(ps, w, x, kc, m, mw, 0, BL)
            if m % 2 == 0:
                nc.scalar.activation(o[:mw, m, :], ps[:mw], act_fn, bias=b[:mw, m:m + 1])
            elif act_fn == AF.Relu:
                nc.vector.tensor_scalar(out=o[:mw, m, :], in0=ps[:mw],
                                        scalar1=b[:mw, m:m + 1], scalar2=0.0,
                                        op0=ALU.add, op1=ALU.max)
            else:
                nc.vector.tensor_scalar(out=o[:mw, m, :], in0=ps[:mw],
                                        scalar1=b[:mw, m:m + 1], scalar2=None,
                                        op0=ALU.add)
        return o

    # ---------- dec MLP -> f_action -> emb[0:3] ----------
    emb = act.tile([16, BL], FP8)
    h = finstT
    for li, (K, M) in enumerate([(1024, 512), (512, 256), (256, 256), (256, 128)]):
        h = mlp_layer(hp_big, h, w_dec[li], b_dec[li], K // 128, M, tag="dec")
    ps5_full = ps_mm.tile([128, 1024], F32, tag="mm")
    ps5 = ps5_full[:, :BL]
    nc.tensor.matmul(ps5[:3], w_dec[4][:, 0, 0:3], h[:, 0, :], start=True, stop=True)
    e_sb = sm.tile([3, BL], F32, tag="e_sb")
    nc.scalar.activation(e_sb, ps5[:3], AF.Exp, bias=b_dec5[:3, 0:1])
    ps_s_full = ps_sm.tile([128, 512], F32, tag="sm")
    ps_s = ps_s_full[:, :BL]
    nc.tensor.matmul(ps_s[:1], ones[:3, 0:1], e_sb[:], start=True, stop=True)
    r_sb = sm.tile([1, BL], F32, tag="r_sb")
    nc.vector.reciprocal(r_sb, ps_s[:1])
    ps_rb_full = ps_sm.tile([128, 512], F32, tag="sm")
    ps_rb = ps_rb_full[:, :BL]
    nc.tensor.matmul(ps_rb[:3], ones[0:1, 0:3], r_sb[:], start=True, stop=True)
    nc.vector.tensor_tensor(out=emb[0:3], in0=e_sb[:], in1=ps_rb[:3], op=ALU.mult)

    # ---------- map MLPs -> c_kT [batch, feat] bf16 (includes b_att1) ----------
    ckT = []
    for k in range(4):
        h1m = mlp_layer(hp_big, finstT, w_map1[k], b_map1[k], 8, 512, tag="map1")
        mk = mlp_layer(hp_big, h1m, w_map2[k], b_map2[k], 4, 256,
                       act_fn=AF.Identity, tag="mk")
        # ckt_t: [128 batch-in-block, nblk, 256 feat], includes b_att1
        ckt_t = act.tile([128, BL // 128, 256], BF16, tag=f"ckT{k}")
        for bblk in range(BL // 128):
            ps_full = ps_sm.tile([128, 512], F32, tag="sm")
            ps = ps_full[:, :256]
            nc.tensor.matmul(ps, mk[:, 0:2, bblk * 128:bblk * 128 + 128],
                             w_att1[k][:, 0:2, 0:256], perf_mode=DR,
                             start=True, stop=False)
            nc.tensor.matmul(ps, ones1[0:1, :128], b_att1r[k][0:1, :],
                             start=False, stop=True)
            if bblk % 2 == 0:
                nc.scalar.copy(ckt_t[:, bblk, :], ps)
            else:
                nc.vector.tensor_copy(ckt_t[:, bblk, :], ps)
        ckT.append(ckt_t)

    # ================= attention =================
    def relu_ps(use_act, out, ps, bias=None):
        """One fused psum->fp8 relu over the full AP on ACT or DVE."""
        if use_act:
            if bias is None:
                nc.scalar.activation(out, ps, AF.Relu)
            else:
                nc.scalar.activation(out, ps, AF.Relu, bias=bias)
        elif bias is None:
            nc.vector.tensor_scalar(out=out, in0=ps, scalar1=0.0, scalar2=None,
                                    op0=ALU.max)
        else:
            nc.vector.tensor_scalar(out=out, in0=ps, scalar1=bias, scalar2=0.0,
                                    op0=ALU.add, op1=ALU.max)

    def att_rt(blk, rt, s4all):
        # two k's in flight: PE never waits on an activation (software pipeline)
        foT = foT_blks[blk]
        col0 = rt * RT
        ps4_full = ps_sm.tile([128, 512], F32, tag="sm")
        ps4 = ps4_full[:, :RT]
        mask_mv = mask128[rt * 32:(rt + 1) * 32].rearrange("p a b -> p (a b)")

        def att1_mm(k):
            ps1_full = ps_mm.tile([128, 1024], F32, tag="mm")
            for m in range(2):
                ps = ps1_full[:, m * RT:(m + 1) * RT]
                nc.tensor.matmul(ps, w_att1[k][:, 2:4, m * 128:(m + 1) * 128],
                                 foT[:, 0:2, col0:col0 + RT], perf_mode=DR,
                                 start=True, stop=False)
                nc.tensor.matmul(ps, ckT[k][rt * 32:(rt + 1) * 32, blk, m * 128:(m + 1) * 128],
                                 mask_mv, start=False, stop=True,
                                 tile_position=(rt * 32, 0))
            return ps1_full

        def att2_mm(k, h1):
            ps2_full = ps_mm.tile([128, 1024], F32, tag="mm")
            for m in range(2):
                nc.tensor.matmul(ps2_full[:, m * RT:(m + 1) * RT],
                                 w_att2[k][:, 0:2, m * 128:(m + 1) * 128],
                                 h1[:, 0:2, :], perf_mode=DR, start=True, stop=True)
            return ps2_full

        def att3_mm(k, h2):
            ps3_full = ps_tr.tile([128, 512], F32, tag="tr")
            ps3 = ps3_full[:, :RT]
            nc.tensor.matmul(ps3, w_att3[k][:, 0:2, 0:128], h2[:, 0:2, :],
                             perf_mode=DR, start=True, stop=True)
            return ps3

        for ka in (0, 2):
            kb = ka + 1
            ps1 = {ka: att1_mm(ka), kb: att1_mm(kb)}
            h1_, h2_, h3_ = {}, {}, {}
            for j, k in enumerate((ka, kb)):
                h1t = h1p.tile([128, 2, RT], FP8, tag="h1")
                h1_[k] = h1t
                relu_ps(j == 0, h1t.rearrange("p c n -> p (c n)"), ps1[k])
            ps2 = {k: att2_mm(k, h1_[k]) for k in (ka, kb)}
            for j, k in enumerate((ka, kb)):
                h2t = h2p.tile([128, 2, RT], FP8, tag="h2")
                h2_[k] = h2t
                relu_ps(j == 1, h2t.rearrange("p c n -> p (c n)"), ps2[k])
            ps3 = {k: att3_mm(k, h2_[k]) for k in (ka, kb)}
            for j, k in enumerate((ka, kb)):
                h3t = h3p.tile([128, RT], FP8, tag="h3")
                h3_[k] = h3t
                relu_ps(j == 0, h3t, ps3[k], bias=b_att3[k][:, 0:1])
            for k in (ka, kb):
                nc.tensor.matmul(ps4[32 * k:32 * k + 1, :], w_att4[k][:, 0, 0:1],
                                 h3_[k], start=True, stop=True,
                                 tile_position=(0, 32 * k))
        nc.vector.tensor_copy(s4all[:, rt, :], ps4)

    def att_post(blk):
        b0 = blk * PB
        bix = blk_state[blk]
        s4all = att_s4[blk]
        sTk = []
        for k in range(4):
            t = sm.tile([128, NB], F32, tag=f"sTk{k}")
            eng = nc.sync if k % 2 == 0 else nc.scalar
            eng.dma_start(out=t[:PB], in_=s4all[32 * k:32 * k + 1, :, :])
            sTk.append(t)

        def softmax_pair(ka, kb, tag):
            z = sm.tile([128, NB], F32, tag=f"z{tag}")
            nc.vector.tensor_tensor(out=z[:PB], in0=sTk[ka][:PB], in1=sTk[kb][:PB],
                                    op=ALU.add)
            e = sm.tile([128, NB], F32, tag=f"e{tag}")
            nc.scalar.activation(e[:PB], z[:PB], AF.Exp)
            ssum = sm.tile([128, 1], F32, tag=f"ss{tag}")
            nc.vector.reduce_sum(ssum[:PB], e[:PB], axis=AX.X)
            rinv = sm.tile([128, 1], F32, tag=f"ri{tag}")
            nc.vector.reciprocal(rinv[:PB], ssum[:PB])
            a = act.tile([128, NB], F32, tag=f"a{tag}{blk}")
            nc.vector.tensor_scalar(out=a[:PB], in0=e[:PB], scalar1=rinv[:PB, 0:1],
                                    scalar2=None, op0=ALU.mult)
            return a

        a_sub = softmax_pair(0, 1, "s")
        a_obj = softmax_pair(2, 3, "o")

        def weighted_loc(a, tag):
            prod = sm.tile([128, 5, NB], F32, tag=f"lp{tag}")
            nc.vector.tensor_tensor(out=prod[:PB], in0=bix[:PB].transpose([0, 2, 1]),
                                    in1=a[:PB].unsqueeze(1).broadcast_to([PB, 5, NB]),
                                    op=ALU.mult)
            loc = sm.tile([128, 5], F32, tag=f"loc{tag}")
            nc.vector.reduce_sum(loc[:PB], prod[:PB], axis=AX.X)
            return loc

        sloc = weighted_loc(a_sub, "s")
        oloc = weighted_loc(a_obj, "o")
        for loc, r0 in ((oloc, 3), (sloc, 8)):
            pt_full = ps_tr.tile([128, 512], F32, tag="tr")
            pt = pt_full[:, :128]
            pe_transpose(nc, pt[:5, :PB], loc[:PB], ident[:PB, :PB])
            locT = sm.tile([5, 128], FP8, tag="locT")
            nc.scalar.copy(locT[:, :PB], pt[:5, :PB])
            nc.sync.dma_start(out=emb[r0:r0 + 5, b0:b0 + PB], in_=locT[:, :PB])
        blk_state[blk] = (a_sub, bix)

    att_s4 = {}
    for blk in range(NBLK):
        s4all = sm.tile([128, NRT_B, RT], F32, tag="s4all")
        att_s4[blk] = s4all
        for rt in range(NRT_B):
            att_rt(blk, rt, s4all)
        att_post(blk)

    # ================= pred MLP + blend =================
    p1 = hp_big.tile([128, 2, BL], FP8, tag="pred")
    for m in range(2):
        ps_full = ps_mm.tile([128, 1024], F32, tag="mm")
        ps = ps_full[:, :BL]
        nc.tensor.matmul(ps, w_pred[0][:13, 0, m * 128:(m + 1) * 128], emb[:13],
                         start=True, stop=True)
        if m == 0:
            nc.scalar.activation(p1[:, m, :], ps, AF.Relu, bias=b_pred[0][:, m:m + 1])
        else:
            nc.vector.tensor_scalar(out=p1[:, m, :], in0=ps,
                                    scalar1=b_pred[0][:, m:m + 1], scalar2=0.0,
                                    op0=ALU.add, op1=ALU.max)
    h = p1
    for li, (K, M) in enumerate([(256, 512), (512, 512), (512, 256)], 1):
        h = mlp_layer(hp_big, h, w_pred[li], b_pred[li], K // 128, M, tag="pred")
    ps5_full = ps_sm.tile([128, 512], F32, tag="sm")
    ps5 = ps5_full[:, :BL]
    for c in range(2):
        nc.tensor.matmul(ps5[:5], w_pred[4][:, c, 0:5], h[:, c, :],
                         start=(c == 0), stop=(c == 1))
    predv = act.tile([5, BL], F32)
    nc.scalar.activation(predv, ps5[:5], AF.Tanh, bias=b_pred5[:5, 0:1])

    for blk in range(NBLK):
        b0 = blk * PB
        a_sub, bix = blk_state[blk]
        pt_full = ps_tr.tile([128, 512], F32, tag="tr")
        pt = pt_full[:, :128]
        pe_transpose(nc, pt[:PB, :5], predv[:, b0:b0 + PB], ident[:5, :5])
        predT = sm.tile([128, 5], F32, tag="predT")
        nc.vector.tensor_copy(predT[:PB], pt[:PB, :5])
        d = scp.tile([128, NB, 5], F32, tag="d")
        nc.vector.tensor_tensor(out=d[:PB], in0=predT[:PB].unsqueeze(1)
                                .broadcast_to([PB, NB, 5]), in1=bix[:PB], op=ALU.subtract)
        nc.vector.tensor_tensor(out=d[:PB], in0=d[:PB],
                                in1=a_sub[:PB].unsqueeze(2).broadcast_to([PB, NB, 5]),
                                op=ALU.mult)
        outb = scp.tile([128, NB, 5], F32, tag="outb")
        nc.vector.tensor_tensor(out=outb[:PB], in0=d[:PB], in1=bix[:PB], op=ALU.add)
        nc.sync.dma_start(out=out_pred[b0:b0 + PB], in_=outb[:PB])

    ctx.close()


INPUT_SPECS = [
    ("f_objects", (NB, DV)), ("f_objects_final", (NB, DV)),
    ("bboxes_i", (NB, 5)), ("bboxes_f", (NB, 5)),
    ("f_instruction", (DL,)), ("gumbel", (NB, NB)),
]
WEIGHT_SPECS = (
    [(f"dec_W{i}", s) for i, s in enumerate([(1024, 512), (512, 256), (256, 256), (256, 128), (128, 3)], 1)]
    + [(f"dec_b{i}", (s,)) for i, s in enumerate([512, 256, 256, 128, 3], 1)]
    + [("map_W1", (4, 1024, 512)), ("map_b1", (4, 512)), ("map_W2", (4, 512, 256)), ("map_b2", (4, 256))]
    + [(f"att_W{i}", (4,) + s) for i, s in enumerate([(512, 256), (256, 256), (256, 128), (128, 1)], 1)]
    + [(f"att_b{i}", (4, s)) for i, s in enumerate([256, 256, 128, 1], 1)]
    + [(f"pred_W{i}", s) for i, s in enumerate([(13, 256), (256, 512), (512, 512), (512, 256), (256, 5)], 1)]
    + [(f"pred_b{i}", (s,)) for i, s in enumerate([256, 512, 512, 256, 5], 1)]
)


def declare_io(nc, BL):
    io = {}
    for name, tail in INPUT_SPECS:
        io[name] = nc.dram_tensor(name, [BL] + list(tail), F32, kind="ExternalInput").ap()
    for name, shape in WEIGHT_SPECS:
        io[name] = nc.dram_tensor(name, list(shape), F32, kind="ExternalInput").ap()
    io["out_pred"] = nc.dram_tensor("out_pred", [BL, NB, 5], F32, kind="ExternalOutput").ap()
    io["out_matched"] = nc.dram_tensor("out_matched", [BL, NB, 5], F32, kind="ExternalOutput").ap()
    return io
# ======================= SPMD driver =======================
import numpy as np

N_CORES = 8
B_FULL = 4096
BL_CORE = B_FULL // N_CORES

_BATCH_INPUTS = ("f_objects", "f_objects_final", "bboxes_i", "bboxes_f",
                 "f_instruction", "gumbel")

_NC = None


def _get_nc():
    global _NC
    if _NC is None:
        from concourse import bacc
        import concourse.tile as tile
        nc = bacc.Bacc("TRN2", target_bir_lowering=False, debug=False,
                       num_devices=N_CORES)
        io = declare_io(nc, BL_CORE)
        with tile.TileContext(nc) as tc:
            build_kernel(tc, io, BL_CORE)
        nc.compile()
        _NC = nc
    return _NC


def kernel(**inputs):
    from concourse.bass_utils import run_bass_kernel_spmd
    nc = _get_nc()
    arrs = {k: np.ascontiguousarray(np.asarray(v, dtype=np.float32))
            for k, v in inputs.items()}
    in_maps = []
    for c in range(N_CORES):
        m = {}
        for k, v in arrs.items():
            if k in _BATCH_INPUTS:
                m[k] = v[c * BL_CORE:(c + 1) * BL_CORE]
            else:
                m[k] = v
        in_maps.append(m)
    res = run_bass_kernel_spmd(nc, in_maps, list(range(N_CORES)))
    pred = np.concatenate([res.results[c]["out_pred"] for c in range(N_CORES)], axis=0)
    matched = np.concatenate([res.results[c]["out_matched"] for c in range(N_CORES)], axis=0)
    return pred, matched


# revision 31
# speedup vs baseline: 1066.6233x; 1.0979x over previous
"""Bass/Tile kernel for nn_BaselineModel (gumbel matching + attention MLPs).

v2 layout/precision plan:
  - MLPs in [feature-on-partition, row-on-free], fp8e4 operands with DoubleRow
    matmuls (K>=256), f32 psum. Matching stays f32 (argmax-flip safety).
  - Per-batch map-branch contribution c_k enters the att1 PSUM group via a
    K=32 one-hot "mask matmul" (batch -> 16 rows), so h1 is a single ACT
    Relu from PSUM (no gpsimd, no DVE broadcast add).
  - att4 scores for the 4 branches land in one PSUM bank at partitions
    {0,32,64,96} (tile_position col groups); one copy + one DMA per (k,blk)
    delivers contiguous [batch, object] score tiles. b_att4 is dropped
    (softmax shift invariance).
  - Matching scan runs on the transposed rho block (free transpose from the
    symmetric Z Z^T gram), so gumbel needs no transpose and every DVE op in
    the scan is contiguous. Matched boxes = accumulated one-hot matrix
    applied post-loop.
"""
import sys
sys.path.insert(0, "/opt/trn_rl_repo")
from contextlib import ExitStack
import concourse.bass as bass
import concourse.mybir as mybir
from concourse.masks import make_identity

F32 = mybir.dt.float32
BF16 = mybir.dt.bfloat16
FP8 = mybir.dt.float8e4
AF = mybir.ActivationFunctionType
ALU = mybir.AluOpType
AX = mybir.AxisListType
DR = mybir.MatmulPerfMode.DoubleRow

NB = 16    # objects per batch
DV = 256   # visual feature dim
DL = 1024  # instruction dim


def ap_view(ap, dims, extra_offset=0):
    return bass.AP(tensor=ap.tensor, offset=ap.offset + extra_offset, ap=list(dims))


def pe_transpose(nc, out, in_, ident):
    return nc.tensor.matmul(out, in_, ident, is_transpose=True, start=True, stop=True)


def build_kernel(tc, io, BL):
    """io: dict name -> DRAM AP (inputs + out_pred, out_matched). BL: batches/core.

    Phase order (v4): input/weight DMAs queue first; gram+scan (weight-
    independent) runs under the DMA stream; then dec/map MLPs (dense PE,
    warms the clock); then attention; then pred+blend.
    """
    nc = tc.nc
    assert BL % 32 == 0
    SUB = 32                    # gram sub-block (batches)
    PB = min(128, BL)           # scan block (batches)
    NBLK = BL // PB
    NSUB_B = PB // SUB          # subs per block
    SUBG = SUB // 8             # groups of 8 batches per sub
    RT = 512                    # rows per attention tile
    BRT = RT // NB              # batches per attention row tile (32)
    NRT_B = PB // BRT           # row tiles per block

    ctx = ExitStack()

    # ---------- pools ----------
    wpool = ctx.enter_context(tc.tile_pool(name="wpool", bufs=1))
    act = ctx.enter_context(tc.tile_pool(name="act", bufs=1))
    hp_big = ctx.enter_context(tc.tile_pool(name="hp_big", bufs=2))
    sm = ctx.enter_context(tc.tile_pool(name="sm", bufs=2))
    scp = ctx.enter_context(tc.tile_pool(name="scp", bufs=2))
    tp = ctx.enter_context(tc.tile_pool(name="tp", bufs=3))
    zp = ctx.enter_context(tc.tile_pool(name="zp", bufs=2))
    h1p = ctx.enter_context(tc.tile_pool(name="h1p", bufs=4))
    h2p = ctx.enter_context(tc.tile_pool(name="h2p", bufs=4))
    h3p = ctx.enter_context(tc.tile_pool(name="h3p", bufs=3))
    fip = ctx.enter_context(tc.tile_pool(name="fip", bufs=1))
    wdec = ctx.enter_context(tc.tile_pool(name="wdec", bufs=1))
    ps_mm = ctx.enter_context(tc.tile_pool(name="ps_mm", bufs=2, space="PSUM"))
    ps_sm = ctx.enter_context(tc.tile_pool(name="ps_sm", bufs=2, space="PSUM"))
    ps_tr = ctx.enter_context(tc.tile_pool(name="ps_tr", bufs=2, space="PSUM"))

    ident = wpool.tile([128, 128], F32)
    make_identity(nc, ident)
    ones = wpool.tile([16, 16], F32)
    nc.vector.memset(ones, 1.0)
    ones1 = wpool.tile([1, 128], FP8)
    nc.vector.memset(ones1, 1.0)
    # one-hot batch->row expansion: mask128[32g + j, b*NB + n] = (b == j)
    mask128 = wpool.tile([128, 32, NB], BF16)
    for g in range(4):
        nc.vector.tensor_copy(
            mask128[g * 32:(g + 1) * 32],
            ident[g * 32:(g + 1) * 32, g * 32:(g + 1) * 32].unsqueeze(2)
            .broadcast_to([32, 32, NB]))

    def load_bias_col(pool, name, M, k=None):
        """DRAM [M] (or [4,M] row k) -> [p, mb] tile, column m = feats m*128..."""
        mb = (M + 127) // 128
        p = min(M, 128)
        t = pool.tile([128, mb], F32, tag=f"b_{name}{'' if k is None else k}")
        off = 0 if k is None else k * M
        nc.scalar.dma_start(out=t[:p, :], in_=ap_view(io[name], [[1, p], [128, mb]], off))
        return t

    def load_bias_row(pool, name, M, k=None):
        """DRAM [M] (or [4,M] row k) -> [1, M] row tile."""
        t = pool.tile([1, M], FP8, tag=f"br_{name}{'' if k is None else k}")
        off = 0 if k is None else k * M
        nc.gpsimd.dma_start(out=t, in_=ap_view(io[name], [[M, 1], [1, M]], off))
        return t

    def load_w(pool, name, K, M, k=None, dt=FP8):
        kc = (K + 127) // 128
        p = min(K, 128)
        t = pool.tile([128, kc, M], dt, tag=f"w_{name}{'' if k is None else k}")
        src = io[name] if k is None else io[name][k]
        view = src.rearrange("(c p) m -> p c m", p=128) if K >= 128 else src.unsqueeze(1)
        nc.gpsimd.dma_start(out=t[:p], in_=view)
        return t

    # ================= emit all input/weight DMAs up front =================
    nbt = max(1, BL // 128)
    btr = min(BL, 128)
    fi_t = []
    for bt in range(nbt):
        fi = fip.tile([128, DL], F32, tag=f"fi{bt}")
        nc.sync.dma_start(out=fi[:btr], in_=io["f_instruction"][bt * 128:bt * 128 + btr])
        fi_t.append(fi)

    w_dec = [load_w(wdec, f"dec_W{i}", K, M) for i, (K, M) in
             enumerate([(1024, 512), (512, 256), (256, 256), (256, 128), (128, 3)], 1)]
    b_dec = [load_bias_col(wdec, f"dec_b{i}", M) for i, M in
             enumerate([512, 256, 256, 128], 1)]
    b_dec5 = load_bias_col(wpool, "dec_b5", 3)
    w_map1 = [load_w(wdec, "map_W1", 1024, 512, k) for k in range(4)]
    w_map2 = [load_w(wdec, "map_W2", 512, 256, k) for k in range(4)]
    b_map1 = [load_bias_col(wdec, "map_b1", 512, k) for k in range(4)]
    b_map2 = [load_bias_col(wdec, "map_b2", 256, k) for k in range(4)]
    w_att1 = [load_w(wpool, "att_W1", 512, 256, k) for k in range(4)]
    w_att2 = [load_w(wpool, "att_W2", 256, 256, k) for k in range(4)]
    w_att3 = [load_w(wpool, "att_W3", 256, 128, k) for k in range(4)]
    w_att4 = [load_w(wpool, "att_W4", 128, 1, k) for k in range(4)]
    b_att1r = [load_bias_row(wpool, "att_b1", 256, k) for k in range(4)]
    b_att2 = [load_bias_col(wpool, "att_b2", 256, k) for k in range(4)]
    b_att3 = [load_bias_col(wpool, "att_b3", 128, k) for k in range(4)]
    w_pred = [load_w(wpool, f"pred_W{i}", K, M) for i, (K, M) in
              enumerate([(13, 256), (256, 512), (512, 512), (512, 256), (256, 5)], 1)]
    b_pred = [load_bias_col(wpool, f"pred_b{i}", M) for i, M in
              enumerate([256, 512, 512, 256], 1)]
    b_pred5 = load_bias_col(wpool, "pred_b5", 5)

    out_pred = io["out_pred"]
    out_matched = io["out_matched"]

    # ================= gram + scan (weight-independent) =================
    foT_blks = []
    blk_tiles = {}
    blk_state = []

    def gram_pre(blk):
        scan_gram = scp.tile([128, 32, 32], F32, tag="scan_gram")
        foT = act.tile([128, 2, PB * NB], FP8, tag=f"foT{blk}")
        foT_blks.append(foT)
        blk_tiles[blk] = (scan_gram, foT)

    def gram_transposes(blk, si):
        scan_gram, foT = blk_tiles[blk]
        s0 = blk * PB + si * SUB
        fo_nat = tp.tile([128, SUBG, 256], F32, tag="fo_nat")
        ff_nat = tp.tile([128, SUBG, 256], F32, tag="ff_nat")
        nc.sync.dma_start(out=fo_nat, in_=io["f_objects"][s0:s0 + SUB]
                          .rearrange("(g b) i d -> (b i) g d", b=8))
        nc.sync.dma_start(out=ff_nat, in_=io["f_objects_final"][s0:s0 + SUB]
                          .rearrange("(g b) i d -> (b i) g d", b=8))
        zt = zp.tile([128, SUB, 2, 32], F32, tag="zt")
        for g in range(SUBG):
            for c in range(2):
                # both transposes share one psum bank: ff -> cols 0:128, fo -> 128:256
                pt_full = ps_tr.tile([128, 512], F32, tag="tr")
                pe_transpose(nc, pt_full[:, 0:128],
                             ff_nat[:, g, c * 128:(c + 1) * 128], ident)
                pe_transpose(nc, pt_full[:, 128:256],
                             fo_nat[:, g, c * 128:(c + 1) * 128], ident)
                nc.vector.tensor_copy(
                    zt[:, g * 8:g * 8 + 8, c, :],
                    ap_view(pt_full, [list(pt_full.ap[0]), [16, 8], [128, 2], [1, 16]]))
                col = (si * SUB + g * 8) * NB
                nc.scalar.copy(foT[:, c, col:col + 128], pt_full[:, 128:256])
        return zt

    def gram_mms(blk, si, zt):
        # gram: 16 batches per psum bank; batch bb -> partitions (bb%4)*32,
        # free cols (bb//4)*32
        scan_gram, _ = blk_tiles[blk]
        gram_sb = sm.tile([128, 2, 128], F32, tag="gram_sb")
        for half in range(2):
            gp_full = ps_sm.tile([128, 512], F32, tag="sm")
            gp = gp_full[:, :128]
            for bb_ in range(16):
                bb = half * 16 + bb_
                po = (bb_ % 4) * 32
                fo_ = (bb_ // 4) * 32
                for c in range(2):
                    nc.tensor.matmul(gp[po:po + 32, fo_:fo_ + 32],
                                     zt[:, bb, c, :], zt[:, bb, c, :],
                                     start=(c == 0), stop=(c == 1),
                                     tile_position=(0, po))
            if half == 0:
                nc.scalar.copy(gram_sb[:, half, :], gp)
            else:
                nc.vector.tensor_copy(gram_sb[:, half, :], gp)
        for q in range(SUB // 4):
            p0 = si * SUB + q * 4
            eng = nc.sync if q % 2 == 0 else nc.scalar
            eng.dma_start(out=scan_gram[p0:p0 + 4],
                          in_=gram_sb[:, q // 4, (q % 4) * 32:(q % 4) * 32 + 32])

    def scan_blk(blk):
        # ---------- matching scan for this block (DVE, transposed base) ----------
        scan_gram, _ = blk_tiles[blk]
        b0 = blk * PB
        gum = scp.tile([128, NB, NB], F32, tag="gum")
        nc.sync.dma_start(out=gum[:PB], in_=io["gumbel"][b0:b0 + PB])
        bfx = scp.tile([128, NB, 5], F32, tag="bfx")
        nc.sync.dma_start(out=bfx[:PB], in_=io["bboxes_f"][b0:b0 + PB])
        bix = act.tile([128, NB, 5], F32, tag=f"bix{blk}")
        nc.sync.dma_start(out=bix[:PB], in_=io["bboxes_i"][b0:b0 + PB])

        gflat = scan_gram.rearrange("p a b -> p (a b)")
        sq_ff = sm.tile([128, NB], F32, tag="sq_ff")
        nc.scalar.activation(sq_ff[:PB], ap_view(gflat, [list(gflat.ap[0]), [33, 16]])[:PB],
                             AF.Sqrt)
        inv_ff = sm.tile([128, NB], F32, tag="inv_ff")
        nc.vector.reciprocal(inv_ff[:PB], sq_ff[:PB])
        sq_fo = sm.tile([128, NB], F32, tag="sq_fo")
        nc.scalar.activation(sq_fo[:PB], ap_view(gflat, [list(gflat.ap[0]), [33, 16]], 528)[:PB],
                             AF.Sqrt)
        inv_fo = sm.tile([128, NB], F32, tag="inv_fo")
        nc.vector.reciprocal(inv_fo[:PB], sq_fo[:PB])

        # baseT[p, i, j] = rho[i,j] + gumbel[i,j]  (rho from gram block [16:32, 0:16])
        baseT = scp.tile([128, NB, NB], F32, tag="baseT")
        nc.vector.tensor_tensor(out=baseT[:PB], in0=scan_gram[:PB, 16:32, 0:16],
                                in1=inv_fo[:PB].unsqueeze(2).broadcast_to([PB, 16, 16]),
                                op=ALU.mult)
        nc.vector.tensor_tensor(out=baseT[:PB], in0=baseT[:PB],
                                in1=inv_ff[:PB].unsqueeze(1).broadcast_to([PB, 16, 16]),
                                op=ALU.mult)
        nc.vector.tensor_tensor(out=baseT[:PB], in0=baseT[:PB], in1=gum[:PB], op=ALU.add)

        pen = sm.tile([128, NB], F32, tag="pen")
        nc.vector.memset(pen[:PB], 0.0)
        pmat = scp.tile([128, NB, NB], F32, tag="pmat")
        for i in range(NB):
            score = sm.tile([128, NB], F32, tag="score")
            nc.vector.tensor_tensor(out=score[:PB], in0=baseT[:PB, i, :], in1=pen[:PB],
                                    op=ALU.add)
            mx8 = sm.tile([128, 8], F32, tag="mx8")
            nc.vector.max(mx8[:PB], score[:PB])
            oh = sm.tile([128, NB], F32, tag="oh")
            nc.vector.tensor_tensor(out=oh[:PB], in0=score[:PB],
                                    in1=mx8[:PB, 0:1].broadcast_to([PB, NB]),
                                    op=ALU.is_equal)
            nc.vector.scalar_tensor_tensor(out=pen[:PB], in0=oh[:PB], scalar=-1e5,
                                           in1=pen[:PB], op0=ALU.mult, op1=ALU.add)
            nc.scalar.copy(pmat[:PB, i, :], oh[:PB])
        boxes = scp.tile([128, NB, 5], F32, tag="boxes")
        for c_ in range(5):
            prod = sm.tile([128, NB, NB], F32, tag="bprod")
            nc.vector.tensor_tensor(out=prod[:PB], in0=pmat[:PB],
                                    in1=bfx[:PB, :, c_].unsqueeze(1)
                                    .broadcast_to([PB, NB, NB]), op=ALU.mult)
            nc.vector.reduce_sum(boxes[:PB, :, c_], prod[:PB], axis=AX.X)
        nc.sync.dma_start(out=out_matched[b0:b0 + PB], in_=boxes[:PB])
        blk_state.append(bix)

    # software-pipeline: transposes of (blk, si) overlap gram MMs of the
    # previous sub, so the PE queue never head-blocks on the zt copy.
    pend = None
    for blk in range(NBLK):
        gram_pre(blk)
        for si in range(NSUB_B):
            zt = gram_transposes(blk, si)
            if pend is not None:
                gram_mms(*pend)
                if pend[1] == NSUB_B - 1:
                    scan_blk(pend[0])
            pend = (blk, si, zt)
    gram_mms(*pend)
    scan_blk(NBLK - 1)

    # ================= dec/map MLPs (dense PE, warms clock) =================
    finstT = act.tile([128, 8, BL], FP8)
    for bt in range(nbt):
        for kc in range(8):
            pt_full = ps_tr.tile([128, 512], F32, tag="tr")
            pt = pt_full[:, :128]
            pe_transpose(nc, pt[:, :btr], fi_t[bt][:btr, kc * 128:(kc + 1) * 128],
                         ident[:btr, :btr])
            if kc % 2 == 0:
                nc.scalar.copy(finstT[:, kc, bt * 128:bt * 128 + btr], pt[:, :btr])
            else:
                nc.vector.tensor_copy(finstT[:, kc, bt * 128:bt * 128 + btr], pt[:, :btr])

    def mm_dr(ps, w, x, kc, m, mw, n_lo, n_hi):
        """psum[:mw, :] += sum_c w[:,c,m128:+mw].T @ x[:,c,n_lo:n_hi], DoubleRow pairs."""
        kc2 = kc // 2
        for c2 in range(kc2):
            nc.tensor.matmul(ps[:mw], w[:, 2 * c2:2 * c2 + 2, m * 128:m * 128 + mw],
                             x[:, 2 * c2:2 * c2 + 2, n_lo:n_hi], perf_mode=DR,
                             start=(c2 == 0), stop=(c2 == kc2 - 1))

    def mlp_layer(pool, x, w, b, kc, M, act_fn=AF.Relu, tag="h"):
        """x: [128, kc, BL] fp8 -> out [128, M/128, BL] fp8 via DR matmuls + ACT."""
        mb = (M + 127) // 128
        o = pool.tile([128, mb, BL], FP8, tag=tag)
        for m in range(mb):
            mw = min(128, M - m * 128)
            ps_full = ps_mm.tile([128, 1024], F32, tag="mm")
            ps = ps_full[:, :BL]
            mm_dr(ps, w, x, kc, m, mw, 0, BL)
            if m % 2 == 0:
                nc.scalar.activation(o[:mw, m, :], ps[:mw], act_fn, bias=b[:mw, m:m + 1])
            elif act_fn == AF.Relu:
                nc.vector.tensor_scalar(out=o[:mw, m, :], in0=ps[:mw],
                                        scalar1=b[:mw, m:m + 1], scalar2=0.0,
                                        op0=ALU.add, op1=ALU.max)
            else:
                nc.vector.tensor_scalar(out=o[:mw, m, :], in0=ps[:mw],
                                        scalar1=b[:mw, m:m + 1], scalar2=None,
                                        op0=ALU.add)
        return o

    # ---------- dec MLP -> f_action -> emb[0:3] ----------
    emb = act.tile([16, BL], FP8)
    h = finstT
    for li, (K, M) in enumerate([(1024, 512), (512, 256), (256, 256), (256, 128)]):
        h = mlp_layer(hp_big, h, w_dec[li], b_dec[li], K // 128, M, tag="dec")
    ps5_full = ps_mm.tile([128, 1024], F32, tag="mm")
    ps5 = ps5_full[:, :BL]
    nc.tensor.matmul(ps5[:3], w_dec[4][:, 0, 0:3], h[:, 0, :], start=True, stop=True)
    e_sb = sm.tile([3, BL], F32, tag="e_sb")
    nc.scalar.activation(e_sb, ps5[:3], AF.Exp, bias=b_dec5[:3, 0:1])
    ps_s_full = ps_sm.tile([128, 512], F32, tag="sm")
    ps_s = ps_s_full[:, :BL]
    nc.tensor.matmul(ps_s[:1], ones[:3, 0:1], e_sb[:], start=True, stop=True)
    r_sb = sm.tile([1, BL], F32, tag="r_sb")
    nc.vector.reciprocal(r_sb, ps_s[:1])
    ps_rb_full = ps_sm.tile([128, 512], F32, tag="sm")
    ps_rb = ps_rb_full[:, :BL]
    nc.tensor.matmul(ps_rb[:3], ones[0:1, 0:3], r_sb[:], start=True, stop=True)
    nc.vector.tensor_tensor(out=emb[0:3], in0=e_sb[:], in1=ps_rb[:3], op=ALU.mult)

    # ---------- map MLPs -> c_kT [batch, feat] bf16 (includes b_att1) ----------
    ckT = []
    for k in range(4):
        h1m = mlp_layer(hp_big, finstT, w_map1[k], b_map1[k], 8, 512, tag="map1")
        mk = mlp_layer(hp_big, h1m, w_map2[k], b_map2[k], 4, 256,
                       act_fn=AF.Identity, tag="mk")
        # ckt_t: [128 batch-in-block, nblk, 256 feat], includes b_att1
        ckt_t = act.tile([128, BL // 128, 256], BF16, tag=f"ckT{k}")
        for bblk in range(BL // 128):
            ps_full = ps_sm.tile([128, 512], F32, tag="sm")
            ps = ps_full[:, :256]
            nc.tensor.matmul(ps, mk[:, 0:2, bblk * 128:bblk * 128 + 128],
                             w_att1[k][:, 0:2, 0:256], perf_mode=DR,
                             start=True, stop=False)
            nc.tensor.matmul(ps, ones1[0:1, :128], b_att1r[k][0:1, :],
                             start=False, stop=True)
            if bblk % 2 == 0:
                nc.scalar.copy(ckt_t[:, bblk, :], ps)
            else:
                nc.vector.tensor_copy(ckt_t[:, bblk, :], ps)
        ckT.append(ckt_t)

    # ================= attention =================
    def relu_ps(use_act, out, ps, bias=None):
        """One fused psum->fp8 relu over the full AP on ACT or DVE."""
        if use_act:
            if bias is None:
                nc.scalar.activation(out, ps, AF.Relu)
            else:
                nc.scalar.activation(out, ps, AF.Relu, bias=bias)
        elif bias is None:
            nc.vector.tensor_scalar(out=out, in0=ps, scalar1=0.0, scalar2=None,
                                    op0=ALU.max)
        else:
            nc.vector.tensor_scalar(out=out, in0=ps, scalar1=bias, scalar2=0.0,
                                    op0=ALU.add, op1=ALU.max)

    def att_rt(blk, rt, s4all):
        # two k's in flight: PE never waits on an activation (software pipeline)
        foT = foT_blks[blk]
        col0 = rt * RT
        ps4_full = ps_sm.tile([128, 512], F32, tag="sm")
        ps4 = ps4_full[:, :RT]
        mask_mv = mask128[rt * 32:(rt + 1) * 32].rearrange("p a b -> p (a b)")

        def att1_mm(k):
            ps1_full = ps_mm.tile([128, 1024], F32, tag="mm")
            for m in range(2):
                ps = ps1_full[:, m * RT:(m + 1) * RT]
                nc.tensor.matmul(ps, w_att1[k][:, 2:4, m * 128:(m + 1) * 128],
                                 foT[:, 0:2, col0:col0 + RT], perf_mode=DR,
                                 start=True, stop=False)
                nc.tensor.matmul(ps, ckT[k][rt * 32:(rt + 1) * 32, blk, m * 128:(m + 1) * 128],
                                 mask_mv, start=False, stop=True,
                                 tile_position=(rt * 32, 0))
            return ps1_full

        def att2_mm(k, h1):
            ps2_full = ps_mm.tile([128, 1024], F32, tag="mm")
            for m in range(2):
                nc.tensor.matmul(ps2_full[:, m * RT:(m + 1) * RT],
                                 w_att2[k][:, 0:2, m * 128:(m + 1) * 128],
                                 h1[:, 0:2, :], perf_mode=DR, start=True, stop=True)
            return ps2_full

        def att3_mm(k, h2):
            ps3_full = ps_tr.tile([128, 512], F32, tag="tr")
            ps3 = ps3_full[:, :RT]
            nc.tensor.matmul(ps3, w_att3[k][:, 0:2, 0:128], h2[:, 0:2, :],
                             perf_mode=DR, start=True, stop=True)
            return ps3

        for ka in (0, 2):
            kb = ka + 1
            ps1 = {ka: att1_mm(ka), kb: att1_mm(kb)}
            h1_, h2_, h3_ = {}, {}, {}
            for j, k in enumerate((ka, kb)):
                h1t = h1p.tile([128, 2, RT], FP8, tag="h1")
                h1_[k] = h1t
                relu_ps(j == 0, h1t.rearrange("p c n -> p (c n)"), ps1[k])
            ps2 = {k: att2_mm(k, h1_[k]) for k in (ka, kb)}
            for j, k in enumerate((ka, kb)):
                h2t = h2p.tile([128, 2, RT], FP8, tag="h2")
                h2_[k] = h2t
                relu_ps(j == 1, h2t.rearrange("p c n -> p (c n)"), ps2[k])
            ps3 = {k: att3_mm(k, h2_[k]) for k in (ka, kb)}
            for j, k in enumerate((ka, kb)):
                h3t = h3p.tile([128, RT], FP8, tag="h3")
                h3_[k] = h3t
                relu_ps(j == 0, h3t, ps3[k], bias=b_att3[k][:, 0:1])
            for k in (ka, kb):
                nc.tensor.matmul(ps4[32 * k:32 * k + 1, :], w_att4[k][:, 0, 0:1],
                                 h3_[k], start=True, stop=True,
                                 tile_position=(0, 32 * k))
        nc.scalar.copy(s4all[:, rt, :], ps4)

    def att_post(blk):
        b0 = blk * PB
        bix = blk_state[blk]
        s4all = att_s4[blk]
        sTk = []
        for k in range(4):
            t = sm.tile([128, NB], F32, tag=f"sTk{k}")
            eng = nc.sync if k % 2 == 0 else nc.scalar
            eng.dma_start(out=t[:PB], in_=s4all[32 * k:32 * k + 1, :, :])
            sTk.append(t)

        def softmax_pair(ka, kb, tag):
            z = sm.tile([128, NB], F32, tag=f"z{tag}")
            nc.vector.tensor_tensor(out=z[:PB], in0=sTk[ka][:PB], in1=sTk[kb][:PB],
                                    op=ALU.add)
            e = sm.tile([128, NB], F32, tag=f"e{tag}")
            nc.scalar.activation(e[:PB], z[:PB], AF.Exp)
            ssum = sm.tile([128, 1], F32, tag=f"ss{tag}")
            nc.vector.reduce_sum(ssum[:PB], e[:PB], axis=AX.X)
            rinv = sm.tile([128, 1], F32, tag=f"ri{tag}")
            nc.vector.reciprocal(rinv[:PB], ssum[:PB])
            a = act.tile([128, NB], F32, tag=f"a{tag}{blk}")
            nc.vector.tensor_scalar(out=a[:PB], in0=e[:PB], scalar1=rinv[:PB, 0:1],
                                    scalar2=None, op0=ALU.mult)
            return a

        a_sub = softmax_pair(0, 1, "s")
        a_obj = softmax_pair(2, 3, "o")

        def weighted_loc(a, tag):
            prod = sm.tile([128, 5, NB], F32, tag=f"lp{tag}")
            nc.vector.tensor_tensor(out=prod[:PB], in0=bix[:PB].transpose([0, 2, 1]),
                                    in1=a[:PB].unsqueeze(1).broadcast_to([PB, 5, NB]),
                                    op=ALU.mult)
            loc = sm.tile([128, 5], F32, tag=f"loc{tag}")
            nc.vector.reduce_sum(loc[:PB], prod[:PB], axis=AX.X)
            return loc

        sloc = weighted_loc(a_sub, "s")
        oloc = weighted_loc(a_obj, "o")
        for loc, r0 in ((oloc, 3), (sloc, 8)):
            pt_full = ps_tr.tile([128, 512], F32, tag="tr")
            pt = pt_full[:, :128]
            pe_transpose(nc, pt[:5, :PB], loc[:PB], ident[:PB, :PB])
            locT = sm.tile([5, 128], FP8, tag="locT")
            nc.scalar.copy(locT[:, :PB], pt[:5, :PB])
            nc.sync.dma_start(out=emb[r0:r0 + 5, b0:b0 + PB], in_=locT[:, :PB])
        blk_state[blk] = (a_sub, bix)

    att_s4 = {}
    for blk in range(NBLK):
        s4all = sm.tile([128, NRT_B, RT], F32, tag="s4all")
        att_s4[blk] = s4all
        for rt in range(NRT_B):
            att_rt(blk, rt, s4all)
        att_post(blk)

    # ================= pred MLP + blend =================
    p1 = hp_big.tile([128, 2, BL], FP8, tag="pred")
    for m in range(2):
        ps_full = ps_mm.tile([128, 1024], F32, tag="mm")
        ps = ps_full[:, :BL]
        nc.tensor.matmul(ps, w_pred[0][:13, 0, m * 128:(m + 1) * 128], emb[:13],
                         start=True, stop=True)
        if m == 0:
            nc.scalar.activation(p1[:, m, :], ps, AF.Relu, bias=b_pred[0][:, m:m + 1])
        else:
            nc.vector.tensor_scalar(out=p1[:, m, :], in0=ps,
                                    scalar1=b_pred[0][:, m:m + 1], scalar2=0.0,
                                    op0=ALU.add, op1=ALU.max)
    h = p1
    for li, (K, M) in enumerate([(256, 512), (512, 512), (512, 256)], 1):
        h = mlp_layer(hp_big, h, w_pred[li], b_pred[li], K // 128, M, tag="pred")
    ps5_full = ps_sm.tile([128, 512], F32, tag="sm")
    ps5 = ps5_full[:, :BL]
    for c in range(2):
        nc.tensor.matmul(ps5[:5], w_pred[4][:, c, 0:5], h[:, c, :],
                         start=(c == 0), stop=(c == 1))
    predv = act.tile([5, BL], F32)
    nc.scalar.activation(predv, ps5[:5], AF.Tanh, bias=b_pred5[:5, 0:1])

    for blk in range(NBLK):
        b0 = blk * PB
        a_sub, bix = blk_state[blk]
        pt_full = ps_tr.tile([128, 512], F32, tag="tr")
        pt = pt_full[:, :128]
        pe_transpose(nc, pt[:PB, :5], predv[:, b0:b0 + PB], ident[:5, :5])
        predT = sm.tile([128, 5], F32, tag="predT")
        nc.vector.tensor_copy(predT[:PB], pt[:PB, :5])
        d = scp.tile([128, NB, 5], F32, tag="d")
        nc.vector.tensor_tensor(out=d[:PB], in0=predT[:PB].unsqueeze(1)
                                .broadcast_to([PB, NB, 5]), in1=bix[:PB], op=ALU.subtract)
        nc.vector.tensor_tensor(out=d[:PB], in0=d[:PB],
                                in1=a_sub[:PB].unsqueeze(2).broadcast_to([PB, NB, 5]),
                                op=ALU.mult)
        outb = scp.tile([128, NB, 5], F32, tag="outb")
        nc.vector.tensor_tensor(out=outb[:PB], in0=d[:PB], in1=bix[:PB], op=ALU.add)
        nc.sync.dma_start(out=out_pred[b0:b0 + PB], in_=outb[:PB])

    ctx.close()


INPUT_SPECS = [
    ("f_objects", (NB, DV)), ("f_objects_final", (NB, DV)),
    ("bboxes_i", (NB, 5)), ("bboxes_f", (NB, 5)),
    ("f_instruction", (DL,)), ("gumbel", (NB, NB)),
]
WEIGHT_SPECS = (
    [(f"dec_W{i}", s) for i, s in enumerate([(1024, 512), (512, 256), (256, 256), (256, 128), (128, 3)], 1)]
    + [(f"dec_b{i}", (s,)) for i, s in enumerate([512, 256, 256, 128, 3], 1)]
    + [("map_W1", (4, 1024, 512)), ("map_b1", (4, 512)), ("map_W2", (4, 512, 256)), ("map_b2", (4, 256))]
    + [(f"att_W{i}", (4,) + s) for i, s in enumerate([(512, 256), (256, 256), (256, 128), (128, 1)], 1)]
    + [(f"att_b{i}", (4, s)) for i, s in enumerate([256, 256, 128, 1], 1)]
    + [(f"pred_W{i}", s) for i, s in enumerate([(13, 256), (256, 512), (512, 512), (512, 256), (256, 5)], 1)]
    + [(f"pred_b{i}", (s,)) for i, s in enumerate([256, 512, 512, 256, 5], 1)]
)


def declare_io(nc, BL):
    io = {}
    for name, tail in INPUT_SPECS:
        io[name] = nc.dram_tensor(name, [BL] + list(tail), F32, kind="ExternalInput").ap()
    for name, shape in WEIGHT_SPECS:
        io[name] = nc.dram_tensor(name, list(shape), F32, kind="ExternalInput").ap()
    io["out_pred"] = nc.dram_tensor("out_pred", [BL, NB, 5], F32, kind="ExternalOutput").ap()
    io["out_matched"] = nc.dram_tensor("out_matched", [BL, NB, 5], F32, kind="ExternalOutput").ap()
    return io
# ======================= SPMD driver =======================
import numpy as np

N_CORES = 8
B_FULL = 4096
BL_CORE = B_FULL // N_CORES

_BATCH_INPUTS = ("f_objects", "f_objects_final", "bboxes_i", "bboxes_f",
                 "f_instruction", "gumbel")

_NC = None


def _get_nc():
    global _NC
    if _NC is None:
        from concourse import bacc
        import concourse.tile as tile
        nc = bacc.Bacc("TRN2", target_bir_lowering=False, debug=False,
                       num_devices=N_CORES)
        io = declare_io(nc, BL_CORE)
        with tile.TileContext(nc) as tc:
            build_kernel(tc, io, BL_CORE)
        nc.compile()
        _NC = nc
    return _NC


def kernel(**inputs):
    from concourse.bass_utils import run_bass_kernel_spmd
    nc = _get_nc()
    arrs = {k: np.ascontiguousarray(np.asarray(v, dtype=np.float32))
            for k, v in inputs.items()}
    in_maps = []
    for c in range(N_CORES):
        m = {}
        for k, v in arrs.items():
            if k in _BATCH_INPUTS:
                m[k] = v[c * BL_CORE:(c + 1) * BL_CORE]
            else:
                m[k] = v
        in_maps.append(m)
    res = run_bass_kernel_spmd(nc, in_maps, list(range(N_CORES)))
    pred = np.concatenate([res.results[c]["out_pred"] for c in range(N_CORES)], axis=0)
    matched = np.concatenate([res.results[c]["out_matched"] for c in range(N_CORES)], axis=0)
    return pred, matched
